# revision 1
# baseline (speedup 1.0000x reference)
"""GIN encoder (3-layer, N=50000, E=800000, D=128) on 8 trn2 NeuronCores.

Strategy (graph/data parallel, hardcoded):
  - Nodes padded to 50176 = 8 cores x 6272 (49 windows x 128). Each core
    owns a contiguous block of 6272 destination nodes.
  - Edges partitioned by destination core; cells keyed by (dst window-pair,
    src-part) where src-part A = first 3072 node-block rows of the source's
    core, B = last 3200. Chunks of 128 edges, padded to a uniform count C
    per cell so all cores run one SPMD program.
  - Per layer: dma_gather z[src] rows (fp16, 256B rows, 1024 idxs/gather =
    the SWDGE ring cap); segment-sum via 0/1-indicator matmuls over 256-slot
    window pairs (fp16 -> fp32 PSUM, one PSUM bank per pair so accumulation
    groups never share a zero region); fused GIN MLP (fp16 weights, fp32
    bias+relu on ACT); an extra transposed per-window matmul makes
    node-major fp16 z for the halo exchange.
  - Halo exchange: two AllGathers per layer (A-part after the first 24
    windows finish, B-part at layer end); the ~500us collective latency
    hides under the next chunk of gather work.
  - dma_gather idxs are int16; the A/B tables are 24576/25600 rows, so
    indices stay in range without further splitting.
"""

import numpy as np

N = 50000
E = 800000
D = 128
L = 3
NCORES = 8
NPAD = 50176             # 8 * 6272
PER_CORE = 6272          # 49 * 128
W = 49                   # windows per core
NPAIR = 25               # window pairs (last is a lone window)
WA = 24                  # windows in part A of the halo exchange
NA = WA * 128            # 3072
NB = PER_CORE - NA       # 3200
GW = 8                   # windows per processing group (4 pair-banks)
GROUPS = [list(range(g, min(g + GW, W))) for g in range(0, W, GW)]
AGA_GROUP = 2            # after this group index, windows 0..23 are done
PAD_SLOT = 300.0         # outside [0, 256) -> zero indicator column
MAXG = 1024              # max idxs per dma_gather (SWDGE ring cap)
C_LONE = None            # chunk count of the lone last pair (set at prep)


def _pair_chunks(C, npairs_in_group):
    return npairs_in_group * C


def _prepare_edges(edge_index):
    """Partition/pad edges -> per-core idx (int16) + dstslot (fp16) arrays.

    Cell = (dst window-pair, src part); dst slot in [0, 256). Flat chunk
    order: (group, part, pair, chunk). Returns (C, Cl, idx_arrs, dst_arrs).
    """
    src = np.asarray(edge_index[0], dtype=np.int64)
    dst = np.asarray(edge_index[1], dtype=np.int64)

    core = dst // PER_CORE
    local = dst % PER_CORE
    pair = local // 256                      # 0..24 (pair 24 = lone window)
    slot = local % 256

    s_core = src // PER_CORE
    s_ln = src % PER_CORE
    part = (s_ln >= NA).astype(np.int64)
    idxval = np.where(part == 0, s_core * NA + s_ln,
                      s_core * NB + (s_ln - NA))

    cell = (core * NPAIR + pair) * 2 + part
    n_cells = NCORES * NPAIR * 2
    counts = np.bincount(cell, minlength=n_cells)
    cc = counts.reshape(NCORES, NPAIR, 2)
    C = int(np.ceil(cc[:, :NPAIR - 1].max() / 128))
    Cl = int(np.ceil(cc[:, NPAIR - 1].max() / 128))

    order = np.lexsort((idxval, cell))
    cell_s = cell[order]
    idx_s = idxval[order]
    slot_s = slot[order]

    caps = np.where(np.arange(n_cells) // 2 % NPAIR == NPAIR - 1,
                    Cl * 128, C * 128)
    cell_starts = np.zeros(n_cells + 1, np.int64)
    np.cumsum(counts, out=cell_starts[1:])
    pos = np.arange(E) - cell_starts[cell_s]
    cap_starts = np.zeros(n_cells + 1, np.int64)
    np.cumsum(caps, out=cap_starts[1:])
    flat = cap_starts[cell_s] + pos

    tot = int(cap_starts[-1])
    idx_flat = np.zeros(tot, np.int64)
    slot_flat = np.full(tot, PAD_SLOT, np.float64)
    idx_flat[flat] = idx_s
    slot_flat[flat] = slot_s

    # per (core, pair, part) ragged blocks -> flat (group, part, pair, chunk)
    idx_arrs, dst_arrs = [], []
    for c in range(NCORES):
        icols, dcols = [], []
        for wins in GROUPS:
            pairs = sorted({w // 2 for w in wins})
            for p in (0, 1):
                blocks_i, blocks_s = [], []
                for pr in pairs:
                    cid = (c * NPAIR + pr) * 2 + p
                    s0, s1 = cap_starts[cid], cap_starts[cid + 1]
                    blocks_i.append(idx_flat[s0:s1])
                    blocks_s.append(slot_flat[s0:s1])
                blk_i = np.concatenate(blocks_i)
                blk_s = np.concatenate(blocks_s).reshape(-1, 128).T
                wrapped = blk_i.reshape(-1, 16).T
                icols.append(np.tile(wrapped, (8, 1)))
                dcols.append(blk_s)
        idx_arrs.append(np.concatenate(icols, axis=1).astype(np.int16))
        dst_arrs.append(np.concatenate(dcols, axis=1).astype(np.float16))
    return C, Cl, idx_arrs, dst_arrs


def _gather_sizes(nch):
    per = MAXG // 128
    return [min(per, nch - k) for k in range(0, nch, per)]


def _build_program(C, Cl, n_devices=NCORES, collectives=True, taps=False):
    import concourse.bacc as bacc
    import concourse.tile as tile
    import concourse.mybir as mybir
    from contextlib import ExitStack

    f32 = mybir.dt.float32
    f16 = mybir.dt.float16
    i16 = mybir.dt.int16
    Relu = mybir.ActivationFunctionType.Relu

    nc = bacc.Bacc("TRN2", debug=False, enable_asserts=False,
                   target_bir_lowering=False, num_devices=n_devices)

    TOTCH = (NPAIR - 1) * 2 * C + 2 * Cl
    TOTIC = TOTCH * 8

    xA_t = nc.dram_tensor("xA", [NCORES * NA, D], f16, kind="ExternalInput")
    xB_t = nc.dram_tensor("xB", [NCORES * NB, D], f16, kind="ExternalInput")
    xT32_t = nc.dram_tensor("xT32", [D, PER_CORE], f32, kind="ExternalInput")
    w1_t = nc.dram_tensor("w1", [D, L * D], f16, kind="ExternalInput")
    w2_t = nc.dram_tensor("w2", [D, L * D], f16, kind="ExternalInput")
    b1_t = nc.dram_tensor("b1", [D, L], f32, kind="ExternalInput")
    b2_t = nc.dram_tensor("b2", [D, L], f32, kind="ExternalInput")
    b2m_t = nc.dram_tensor("b2mat", [D, L * D], f32, kind="ExternalInput")
    iota_t = nc.dram_tensor("iota", [D, (MAXG // 128) * 256], f16,
                            kind="ExternalInput")
    idx_t = nc.dram_tensor("idx", [128, TOTIC], i16, kind="ExternalInput")
    dst_t = nc.dram_tensor("dsts", [128, TOTCH], f16, kind="ExternalInput")
    zout_t = nc.dram_tensor("zout", [D, PER_CORE], f32, kind="ExternalOutput")
    if taps:
        agg_o = nc.dram_tensor("agg_o", [128, GW * 128], f32,
                               kind="ExternalOutput")

    rg = [list(range(NCORES))]

    with tile.TileContext(nc) as tc, ExitStack() as ctx:
        const = ctx.enter_context(tc.tile_pool(name="const", bufs=1))
        ztp = ctx.enter_context(tc.tile_pool(name="zt", bufs=1))
        gp = ctx.enter_context(tc.tile_pool(name="g", bufs=4))
        mp = ctx.enter_context(tc.tile_pool(name="m", bufs=3))
        hp = ctx.enter_context(tc.tile_pool(name="h", bufs=2))
        zbp = ctx.enter_context(tc.tile_pool(name="zb", bufs=2))
        aggp = ctx.enter_context(tc.tile_pool(name="agg", bufs=4, space="PSUM"))
        p1p = ctx.enter_context(tc.tile_pool(name="p1", bufs=2, space="PSUM"))
        p2p = ctx.enter_context(tc.tile_pool(name="p2", bufs=2, space="PSUM"))
        dram = ctx.enter_context(tc.tile_pool(name="dram", bufs=1, space="DRAM"))

        w1s = const.tile([D, L * D], f16)
        w2s = const.tile([D, L * D], f16)
        b1s = const.tile([D, L], f32)
        b2s = const.tile([D, L], f32)
        b2ms = const.tile([D, L * D], f32)
        iotas = const.tile([D, (MAXG // 128) * 256], f16)
        idxs = const.tile([128, TOTIC], i16)
        dsts = const.tile([128, TOTCH], f16)
        for sb, t in ((w1s, w1_t), (w2s, w2_t), (b1s, b1_t), (b2s, b2_t),
                      (b2ms, b2m_t), (iotas, iota_t), (idxs, idx_t),
                      (dsts, dst_t)):
            nc.sync.dma_start(sb[:], t.ap())

        ztA = ztp.tile([D, PER_CORE], f32)
        ztB = ztp.tile([D, PER_CORE], f32)
        z16 = ztp.tile([128, W * 128], f16)
        nc.sync.dma_start(ztA[:], xT32_t.ap())
        z16r = z16.rearrange("p (w d) -> p w d", d=128)

        zblkA = [dram.tile([NA, D], f16, name=f"zblkA{l}", tag=f"zblkA{l}")
                 for l in range(L - 1)]
        zblkB = [dram.tile([NB, D], f16, name=f"zblkB{l}", tag=f"zblkB{l}")
                 for l in range(L - 1)]
        sh = "Shared" if collectives else "Local"
        zshA = [dram.tile([NCORES * NA, D], f16, addr_space=sh,
                          name=f"zshA{l}", tag=f"zshA{l}") for l in range(L - 1)]
        zshB = [dram.tile([NCORES * NB, D], f16, addr_space=sh,
                          name=f"zshB{l}", tag=f"zshB{l}") for l in range(L - 1)]

        def halo(l, blk, shr, z16slice):
            nc.sync.dma_start(
                blk.rearrange("(w p) d -> p w d", p=128), z16slice)
            if collectives:
                nc.gpsimd.collective_compute(
                    "AllGather", mybir.AluOpType.bypass, replica_groups=rg,
                    ins=[blk.opt()], outs=[shr.opt()])
            else:
                nc.sync.dma_start(
                    shr.rearrange("(r n) d -> r n d", r=NCORES)[0], blk[:])

        for l in range(L):
            zt_cur = ztA if l % 2 == 0 else ztB
            zt_next = ztB if l % 2 == 0 else ztA
            srcs = [xA_t.ap(), xB_t.ap()] if l == 0 else \
                   [zshA[l - 1][:], zshB[l - 1][:]]

            icol = 0
            ccol = 0
            for gi, wins in enumerate(GROUPS):
                wg = len(wins)
                nn = wg * 128
                n0 = wins[0] * 128
                pairs = sorted({w // 2 for w in wins})
                cC = [Cl if pr == NPAIR - 1 else C for pr in pairs]
                cum = np.cumsum([0] + cC)
                aggs = [aggp.tile([128, 256], f32, tag="aggw",
                                  name=f"agg_l{l}g{gi}p{pi}")
                        for pi in range(len(pairs))]

                for p in (0, 1):
                    ch0 = 0
                    for nchk in _gather_sizes(int(cum[-1])):
                        gb = gp.tile([128, MAXG // 128, 128], f16, tag="g")
                        nc.gpsimd.dma_gather(
                            gb[:, 0:nchk, :], srcs[p],
                            idxs[:, icol:icol + nchk * 8],
                            nchk * 128, nchk * 128, 128,
                        )
                        mb = mp.tile([128, MAXG // 128, 256], f16, tag="m")
                        dst3 = dsts[:, ccol:ccol + nchk].to_broadcast(
                            (128, nchk, 256))
                        iota3 = iotas[:, 0:nchk * 256].rearrange(
                            "p (c f) -> p c f", f=256)
                        nc.vector.tensor_tensor(
                            mb[:, 0:nchk, :], iota3, dst3,
                            op=mybir.AluOpType.is_equal)
                        for k in range(nchk):
                            fc = ch0 + k
                            pi = int(np.searchsorted(cum, fc, side="right")) - 1
                            c = fc - int(cum[pi])
                            nc.tensor.matmul(
                                aggs[pi][:],
                                lhsT=gb[:, k, :], rhs=mb[:, k, :],
                                start=(p == 0 and c == 0),
                                stop=(p == 1 and c == cC[pi] - 1),
                            )
                        icol += nchk * 8
                        ccol += nchk
                        ch0 += nchk

                if taps and l == 0 and gi == 0:
                    aggt = hp.tile([128, GW * 128], f32, tag="aggtap")
                    for pi in range(len(pairs)):
                        nslots = 256 if pairs[pi] != NPAIR - 1 else 128
                        nc.vector.tensor_copy(
                            aggt[:, pi * 256:pi * 256 + nslots],
                            aggs[pi][:, 0:nslots])
                    nc.sync.dma_start(agg_o.ap(), aggt[:])

                # ---- GIN MLP over this group's nodes ------------------
                ht = hp.tile([128, GW * 128], f16, tag="ht")
                for pi in range(len(pairs)):
                    nslots = 256 if pairs[pi] != NPAIR - 1 else 128
                    nc.vector.tensor_add(
                        ht[:, pi * 256:pi * 256 + nslots],
                        aggs[pi][:, 0:nslots],
                        zt_cur[:, n0 + pi * 256:n0 + pi * 256 + nslots])
                h1 = hp.tile([128, GW * 128], f16, tag="h1")
                for s0 in range(0, nn, 512):
                    s1 = min(s0 + 512, nn)
                    p1 = p1p.tile([128, 512], f32, tag="p1")
                    nc.tensor.matmul(p1[:, 0:s1 - s0],
                                     lhsT=w1s[:, l * D:(l + 1) * D],
                                     rhs=ht[:, s0:s1])
                    nc.scalar.activation(h1[:, s0:s1], p1[:, 0:s1 - s0],
                                         Relu, bias=b1s[:, l:l + 1])
                    p2 = p2p.tile([128, 512], f32, tag="p2")
                    nc.tensor.matmul(p2[:, 0:s1 - s0],
                                     lhsT=w2s[:, l * D:(l + 1) * D],
                                     rhs=h1[:, s0:s1])
                    nc.scalar.activation(zt_next[:, n0 + s0:n0 + s1],
                                         p2[:, 0:s1 - s0],
                                         Relu, bias=b2s[:, l:l + 1])
                if l < L - 1:
                    for wi in range(wg):
                        wa = wins[wi]
                        p2b = p1p.tile([128, 512], f32, tag="p1")
                        nc.tensor.matmul(
                            p2b[:, 0:128],
                            lhsT=h1[:, wi * 128:(wi + 1) * 128],
                            rhs=w2s[:, l * D:(l + 1) * D])
                        zb = zbp.tile([128, 128], f32, tag="zbt")
                        nc.vector.tensor_add(zb[:], p2b[:, 0:128],
                                             b2ms[:, l * D:(l + 1) * D])
                        nc.vector.tensor_scalar_max(z16r[:, wa, :], zb[:], 0.0)

                    if gi == AGA_GROUP:
                        halo(l, zblkA[l], zshA[l], z16r[:, 0:WA, :])

            if l < L - 1:
                halo(l, zblkB[l], zshB[l], z16r[:, WA:W, :])

        nc.sync.dma_start(zout_t.ap(), ztB[:])

    nc.compile()
    return nc


def make_in_maps(inputs, C, Cl, idx_arrs, dst_arrs):
    x = np.asarray(inputs["x"], dtype=np.float32)
    Ws1 = np.asarray(inputs["Ws1"], dtype=np.float32)
    bs1 = np.asarray(inputs["bs1"], dtype=np.float32)
    Ws2 = np.asarray(inputs["Ws2"], dtype=np.float32)
    bs2 = np.asarray(inputs["bs2"], dtype=np.float32)

    x_pad = np.zeros((NPAD, D), np.float32)
    x_pad[:N] = x
    xg16 = x_pad.astype(np.float16).reshape(NCORES, PER_CORE, D)
    xA = np.ascontiguousarray(xg16[:, :NA].reshape(NCORES * NA, D))
    xB = np.ascontiguousarray(xg16[:, NA:].reshape(NCORES * NB, D))
    w1 = np.concatenate([Ws1[l] for l in range(L)], axis=1).astype(np.float16)
    w2 = np.concatenate([Ws2[l] for l in range(L)], axis=1).astype(np.float16)
    b1 = np.ascontiguousarray(bs1.T).astype(np.float32)
    b2 = np.ascontiguousarray(bs2.T).astype(np.float32)
    b2mat = np.concatenate(
        [np.broadcast_to(bs2[l][None, :], (D, D)) for l in range(L)],
        axis=1).astype(np.float32)
    iota = np.broadcast_to(
        np.tile(np.arange(256, dtype=np.float16), MAXG // 128)[None, :],
        (D, (MAXG // 128) * 256)).astype(np.float16)

    in_maps = []
    for c in range(NCORES):
        xT32 = np.ascontiguousarray(
            x_pad[c * PER_CORE:(c + 1) * PER_CORE].T)
        in_maps.append({
            "xA": xA, "xB": xB, "xT32": xT32, "w1": w1, "w2": w2,
            "b1": b1, "b2": b2, "b2mat": b2mat, "iota": iota,
            "idx": idx_arrs[c], "dsts": dst_arrs[c],
        })
    return in_maps


def kernel(x, Ws1, bs1, Ws2, bs2, edge_index):
    C, Cl, idx_arrs, dst_arrs = _prepare_edges(edge_index)
    in_maps = make_in_maps(
        {"x": x, "Ws1": Ws1, "bs1": bs1, "Ws2": Ws2, "bs2": bs2},
        C, Cl, idx_arrs, dst_arrs)

    nc = _build_program(C, Cl)

    from concourse.bass_utils import run_bass_kernel_spmd
    res = run_bass_kernel_spmd(nc, in_maps, core_ids=list(range(NCORES)))
    global last_results
    last_results = res

    out = np.empty((NPAD, D), np.float32)
    for c in range(NCORES):
        out[c * PER_CORE:(c + 1) * PER_CORE] = res.results[c]["zout"].T
    return out[:N]



# revision 23
# speedup vs baseline: 1.5347x; 1.5347x over previous
"""GIN encoder (3-layer, N=50000, E=800000, D=128) on 8 trn2 NeuronCores.

v2 strategy — descriptor-free aggregation (no dma_gather):
  - Every core keeps the FULL node-feature table Z in SBUF, node-major
    bf16 [128 slots, 392 windows, 128 feat] (all-gathered per layer).
  - Edges partitioned by dst core; per core the edge stream is grouped
    into cells (parity(dst), src window), padded uniformly across cores
    (SPMD). Per 512-column tile:
      1. PE "broadcast" matmul (one-hot lhsT E_k) replicates the tile's
         per-edge src-slot values from a packed [128, *] table to all
         128 partitions (PSUM fp32).
      2. DVE is_equal vs a per-partition iota builds the slot indicator
         [slot, col] in bf16.
      3. One PE matmul per (window-run in tile) gathers z[src] columns:
         G[feat, col] = Z_win^T_slotmajor @ indicator  (PSUM fp32).
      4. ACT copies G into a staging ring, bf16, stride-2 (d=2 layout
         with a permanent-zero partner slot).
      5. gpsimd.scatter_add accumulates staging into the feature-major
         agg [128, npairs, 2] (bf16), idx = dst node-pair; the odd-dst
         pass uses a one-column-shifted view of the same agg buffer.
    scatter_add loses duplicate updates within an aligned 8-index octet
    (SIMD width 8), so same-pair edges are round-robined across octets
    per cell at prep time; pad columns add 0 to a dump pair.
  - MLP runs feature-major on [128, 6272] (h = agg + z), then the own
    z_next is PE-transposed to node-major, DMA'd to HBM and AllGathered
    for the next layer's Z table.
"""

import numpy as np

N = 50000
E = 800000
D = 128
L = 3
NCORES = 8
PER_CORE = 6272          # 49 * 128 dst nodes per core
NPAD = 50176             # 8 * 6272
NW = 392                 # global 128-node source windows
NWC = 49                 # windows per core
NPAIRS = 3136            # dst node pairs per core
TILE = 512               # column tile (one PSUM bank)
BATCH = 3584             # scatter_add batch = 7 tiles, %16 == 0
NELEMS = 3140            # scatter_add num_elems (3136 real + dump space)
DUMP = 3139              # dump pair for pad columns
PADV = 300.0             # src-slot value for pad columns (never matches)
EBLK = 32                # one-hot broadcast matrices E_0..E_63


def _prepare_edges(edge_index):
    """Build the uniform cell geometry + per-core tables.

    Returns (geom, percore) where geom has the shared static structure and
    percore the per-core srcvals/idx tables.
    """
    src = np.asarray(edge_index[0], dtype=np.int64)
    dst = np.asarray(edge_index[1], dtype=np.int64)

    core = dst // PER_CORE
    dloc = dst % PER_CORE
    par = dloc & 1
    w = src >> 7
    slot = src & 127
    pairv = np.where(par == 0, dloc >> 1, (dloc - 1) >> 1)

    # cell id per edge: (core, par, w)
    cell = (core * 2 + par) * NW + w
    ncells = NCORES * 2 * NW
    counts = np.bincount(cell, minlength=ncells)

    # max multiplicity of (cell, pair) — octet lower bound
    cp = cell * np.int64(NPAIRS) + pairv
    _, cp_counts = np.unique(cp, return_counts=True)
    cp_cell = np.unique(cp) // NPAIRS
    maxmult = np.zeros(ncells, np.int64)
    np.maximum.at(maxmult, cp_cell, cp_counts)

    # scatter_add loses duplicate-pair updates closer than ~4 idx columns
    # (two 8-lane units stream the columns with skew). Enforce SEP-position
    # separation between same-pair updates within a scatter batch.
    SEP = 96

    order = np.lexsort((pairv, cell))
    cell_s = cell[order]
    cell_starts = np.zeros(ncells + 1, np.int64)
    np.cumsum(counts, out=cell_starts[1:])

    # per-cell edge lists (sorted by pair) per core
    K = np.maximum((counts + 7) // 8, maxmult).reshape(NCORES, 2, NW).max(0)

    def place(K):
        """Greedy octet assignment honoring SEP. Returns (ok, needK,
        srcvals, idxvals, off, tot, base)."""
        P = K * 8
        off = np.zeros((2, NW), np.int64)
        tot = np.zeros(2, np.int64)
        for p in (0, 1):
            off[p] = np.cumsum(np.concatenate([[0], P[p][:-1]]))
            tot[p] = int(np.ceil(P[p].sum() / BATCH)) * BATCH
        base = np.array([0, tot[0]], np.int64)
        TOTC = int(tot.sum())
        srcvals = np.full((NCORES, TOTC), PADV, np.float64)
        idxvals = np.full((NCORES, TOTC), DUMP, np.int64)
        needK = K.copy()
        ok = True
        sepo = (SEP + 7) // 8
        for c in range(NCORES):
            for p in (0, 1):
                nextpos = {}
                for wi in range(NW):
                    cid = (c * 2 + p) * NW + wi
                    s0, s1 = cell_starts[cid], cell_starts[cid + 1]
                    if s0 == s1:
                        continue
                    kk = int(K[p, wi])
                    cbase = int(base[p] + off[p, wi])
                    cap = np.zeros(kk, np.int64)
                    fill = [[] for _ in range(kk)]
                    eidx = order[s0:s1]
                    prs = pairv[eidx]
                    # groups by pair, biggest first
                    upr, inv, cnt = np.unique(prs, return_inverse=True,
                                              return_counts=True)
                    gorder = np.argsort(-cnt)
                    failed = False
                    for gi in gorder:
                        members = eidx[inv == gi]
                        pr = int(upr[gi])
                        for e in members:
                            o0 = nextpos.get(pr, -10**9)
                            o0 = max(0, (o0 - cbase + 7) // 8)
                            o = o0
                            while o < kk and cap[o] >= 8:
                                o += 1
                            if o >= kk:
                                failed = True
                                needK[p, wi] = max(needK[p, wi],
                                                   kk + max(1, o0 - kk + 1))
                                continue
                            fill[o].append(e)
                            cap[o] += 1
                            nextpos[pr] = cbase + 8 * o + 8 + SEP
                    if failed:
                        ok = False
                        continue
                    for o in range(kk):
                        for li, e in enumerate(fill[o]):
                            col = cbase + 8 * o + li
                            srcvals[c, col] = slot[e]
                            idxvals[c, col] = pairv[e]
        return ok, needK, srcvals, idxvals, off, tot, base

    for _ in range(6):
        ok, needK, srcvals, idxvals, off, tot, base = place(K)
        if ok:
            break
        K = needK
    assert ok, "greedy octet placement failed"
    P = K * 8
    TOTC = int(tot.sum())
    assert TOTC % BATCH == 0
    ntiles = TOTC // TILE

    # verify: same-pair separation >= SEP within each batch, octets distinct
    for c in range(NCORES):
        for p in (0, 1):
            covs = np.arange(base[p], base[p] + tot[p])
            idb = idxvals[c, covs].reshape(-1, BATCH)
            for b in range(idb.shape[0]):
                row = idb[b]
                real = row != DUMP
                pos = np.arange(BATCH)[real]
                prs = row[real]
                o = np.lexsort((pos, prs))
                same = prs[o][1:] == prs[o][:-1]
                gap = pos[o][1:] - pos[o][:-1]
                assert not (same & (gap < SEP)).any(), "separation violated"

    # tile segments: per tile, runs of (w, a, b) in-tile col ranges
    # (uniform across cores). Pad ranges use window 0 (indicator all-zero).
    bounds = []              # (colstart, colend, w) in stream order
    for p in (0, 1):
        cstart = base[p]
        for wi in range(NW):
            if P[p, wi]:
                s0 = base[p] + off[p, wi]
                bounds.append((s0, s0 + P[p, wi], wi))
        pe = base[p] + P[p].sum()
        if tot[p] > P[p].sum():
            bounds.append((pe, base[p] + tot[p], 0))
    segs = [[] for _ in range(ntiles)]
    for (s0, s1, wi) in bounds:
        t0, t1 = s0 // TILE, (s1 - 1) // TILE
        for t in range(t0, t1 + 1):
            a = max(s0, t * TILE) - t * TILE
            b = min(s1, (t + 1) * TILE) - t * TILE
            segs[t].append((wi, int(a), int(b)))

    geom = {
        "TOTC": TOTC, "ntiles": ntiles, "segs": segs,
        "tot": tot, "base": base,
        "nbatch": TOTC // BATCH,
        "par_of_tile": [0 if t * TILE < tot[0] else 1 for t in range(ntiles)],
    }
    percore = {"srcvals": srcvals, "idxvals": idxvals}
    return geom, percore


def _pack_tables(geom, percore):
    """srcW packing + wrapped scatter idx tables, per core."""
    TOTC, ntiles = geom["TOTC"], geom["ntiles"]
    nblk = (ntiles + EBLK - 1) // EBLK
    srcw = np.zeros((NCORES, 128, nblk * TILE), np.float64)
    sv = percore["srcvals"].reshape(NCORES, ntiles, TILE)
    for t in range(ntiles):
        srcw[:, t % EBLK, (t // EBLK) * TILE:(t // EBLK + 1) * TILE] = sv[:, t]

    idx = percore["idxvals"].astype(np.int16)    # [NCORES, TOTC]
    nb = geom["nbatch"]
    iw = idx.reshape(NCORES, nb, BATCH // 16, 16)
    idxt = np.tile(iw.transpose(0, 3, 1, 2).reshape(NCORES, 16, nb * (BATCH // 16)),
                   (1, 8, 1))                    # [NCORES, 128, nb*224]
    return srcw, idxt, nblk


def _numpy_sim(inputs, geom, percore):
    """Bit-approximate pipeline sim (fp32 math) to validate the tables."""
    x = np.asarray(inputs["x"], np.float32)
    Ws1, bs1 = np.asarray(inputs["Ws1"], np.float32), np.asarray(inputs["bs1"], np.float32)
    Ws2, bs2 = np.asarray(inputs["Ws2"], np.float32), np.asarray(inputs["bs2"], np.float32)
    xp = np.zeros((NPAD, D), np.float32)
    xp[:N] = x
    z = xp.copy()
    sv = percore["srcvals"]
    iv = percore["idxvals"]
    tot, base = geom["tot"], geom["base"]
    for l in range(L):
        zn = np.zeros_like(z)
        for c in range(NCORES):
            agg2 = np.zeros((D, NELEMS + 1, 2), np.float32)
            # gather G columns
            segs = geom["segs"]
            G = np.zeros((D, geom["TOTC"]), np.float32)
            for t, seglist in enumerate(segs):
                for (wi, a, b) in seglist:
                    cols = np.arange(t * TILE + a, t * TILE + b)
                    s = sv[c, cols]
                    real = s < 128
                    gsl = np.zeros((D, len(cols)), np.float32)
                    nodes = wi * 128 + s[real].astype(np.int64)
                    gsl[:, real] = z[nodes].T
                    G[:, cols] = gsl
            # scatter (true accumulation; octet constraint already asserted)
            for p in (0, 1):
                cols = np.arange(base[p], base[p] + tot[p])
                idxs = iv[c, cols]
                tgt = np.zeros((NELEMS + 1, D), np.float32)
                np.add.at(tgt, idxs, G[:, cols].T)
                agg2[:, :, p] += tgt.T
            # unpack agg2 -> agg cols: even pass wrote (pair k -> col 2k),
            # odd pass wrote (pair k -> col 2k+1)
            agg = np.zeros((D, PER_CORE), np.float32)
            agg[:, 0::2] = agg2[:, :NPAIRS, 0]
            agg[:, 1::2] = agg2[:, :NPAIRS, 1]
            zc = z[c * PER_CORE:(c + 1) * PER_CORE].T
            h = agg + zc
            h1 = np.maximum(Ws1[l].T @ h + bs1[l][:, None], 0)
            z2 = np.maximum(Ws2[l].T @ h1 + bs2[l][:, None], 0)
            zn[c * PER_CORE:(c + 1) * PER_CORE] = z2.T
        z = zn
    return z[:N]


def _build_program(geom, n_devices=NCORES, collectives=True, taps=False):
    import concourse.bacc as bacc
    import concourse.tile as tile
    import concourse.mybir as mybir
    from contextlib import ExitStack

    f32 = mybir.dt.float32
    bf16 = mybir.dt.bfloat16
    i16 = mybir.dt.int16
    Relu = mybir.ActivationFunctionType.Relu
    iseq = mybir.AluOpType.is_equal

    ntiles = geom["ntiles"]
    segs = geom["segs"]
    nb = geom["nbatch"]
    nblk = (ntiles + EBLK - 1) // EBLK
    TPB = BATCH // TILE          # tiles per scatter batch (7)
    IPB = BATCH // 16            # idx cols per batch (224)

    nc = bacc.Bacc("TRN2", debug=False, enable_asserts=False,
                   target_bir_lowering=False, num_devices=n_devices)

    zall0_t = nc.dram_tensor("zall0", [128, NW * 128], bf16, kind="ExternalInput")
    zfm0_t = nc.dram_tensor("zfm0", [128, PER_CORE], bf16, kind="ExternalInput")
    srcw_t = nc.dram_tensor("srcw", [128, nblk * TILE], bf16, kind="ExternalInput")
    emat_t = nc.dram_tensor("emat", [128, EBLK * 128], bf16, kind="ExternalInput")
    iota_t = nc.dram_tensor("iota", [128, 1], f32, kind="ExternalInput")
    ident_t = nc.dram_tensor("ident", [128, 128], bf16, kind="ExternalInput")
    idxt_t = nc.dram_tensor("idxt", [128, nb * IPB], i16, kind="ExternalInput")
    w1_t = nc.dram_tensor("w1", [128, L * 128], bf16, kind="ExternalInput")
    w2_t = nc.dram_tensor("w2", [128, L * 128], bf16, kind="ExternalInput")
    b1_t = nc.dram_tensor("b1", [128, L], f32, kind="ExternalInput")
    b2_t = nc.dram_tensor("b2", [128, L], f32, kind="ExternalInput")
    zout_t = nc.dram_tensor("zout", [128, PER_CORE], f32, kind="ExternalOutput")
    if taps:
        agg_o = nc.dram_tensor("agg_o", [128, 2 * NELEMS + 1], bf16,
                               kind="ExternalOutput")
        z1_o = nc.dram_tensor("z1_o", [128, PER_CORE], bf16,
                              kind="ExternalOutput")
        g_o = nc.dram_tensor("g_o", [128, 4 * TILE], f32,
                             kind="ExternalOutput")
        stg_o = nc.dram_tensor("stg_o", [128, geom["nbatch"] * BATCH * 2],
                               bf16, kind="ExternalOutput")

    rg = [list(range(NCORES))]

    with tile.TileContext(nc) as tc, ExitStack() as ctx:
        const = ctx.enter_context(tc.tile_pool(name="const", bufs=1))
        zap = ctx.enter_context(tc.tile_pool(name="za", bufs=1))
        zfp = ctx.enter_context(tc.tile_pool(name="zf", bufs=1))
        agp = ctx.enter_context(tc.tile_pool(name="ag", bufs=1))
        stp = ctx.enter_context(tc.tile_pool(name="st", bufs=1))
        indp = ctx.enter_context(tc.tile_pool(name="ind", bufs=2))
        smallp = ctx.enter_context(tc.tile_pool(name="sm", bufs=2))
        bcp = ctx.enter_context(tc.tile_pool(name="bc", bufs=2, space="PSUM"))
        gpp = ctx.enter_context(tc.tile_pool(name="gp", bufs=2, space="PSUM"))
        mlpp = ctx.enter_context(tc.tile_pool(name="mlp", bufs=2, space="PSUM"))
        tpp = ctx.enter_context(tc.tile_pool(name="tp", bufs=2, space="PSUM"))
        dram = ctx.enter_context(tc.tile_pool(name="dram", bufs=1, space="DRAM"))

        srcw = const.tile([128, nblk * TILE], bf16)
        emat = const.tile([128, EBLK * 128], bf16)
        iota = const.tile([128, 1], f32)
        ident = const.tile([128, 128], bf16)
        idxt = const.tile([128, nb * IPB], i16)
        w1 = const.tile([128, L * 128], bf16)
        w2 = const.tile([128, L * 128], bf16)
        b1 = const.tile([128, L], f32)
        b2 = const.tile([128, L], f32)
        for sb, t in ((srcw, srcw_t), (emat, emat_t), (iota, iota_t),
                      (ident, ident_t), (idxt, idxt_t), (w1, w1_t),
                      (w2, w2_t), (b1, b1_t), (b2, b2_t)):
            nc.sync.dma_start(sb[:], t.ap())

        zall = zap.tile([128, NW, 128], bf16)
        nc.sync.dma_start(zall.rearrange("p w d -> p (w d)"), zall0_t.ap())
        zfmA = zfp.tile([128, PER_CORE], bf16)
        zfmB = zfp.tile([128, PER_CORE], bf16)
        nc.sync.dma_start(zfmA[:], zfm0_t.ap())
        agg = agp.tile([128, 2 * NELEMS + 1], bf16)
        stgs = [stp.tile([128, BATCH, 2], bf16, name=f"stg{i}") for i in (0, 1)]
        for s in stgs:
            nc.vector.memset(s.rearrange("p e two -> p (e two)"), 0.0)

        zblk = [dram.tile([PER_CORE, 128], bf16, name=f"zblk{l}", tag=f"zblk{l}")
                for l in range(L - 1)]
        sh = "Shared" if collectives else "Local"
        zsh = [dram.tile([NPAD, 128], bf16, addr_space=sh,
                         name=f"zsh{l}", tag=f"zsh{l}") for l in range(L - 1)]

        for l in range(L):
            zfm_cur = zfmA if l % 2 == 0 else zfmB
            zfm_nxt = zfmB if l % 2 == 0 else zfmA
            nc.vector.memset(agg[:], 0.0)

            for t in range(ntiles):
                par = geom["par_of_tile"][t]
                bc = bcp.tile([128, TILE], f32, tag="bc")
                nc.tensor.matmul(
                    bc[:], lhsT=emat[:, (t % EBLK) * 128:(t % EBLK + 1) * 128],
                    rhs=srcw[:, (t // EBLK) * TILE:(t // EBLK + 1) * TILE],
                    start=True, stop=True)
                ind = indp.tile([128, TILE], bf16, tag="ind")
                nc.vector.tensor_tensor(
                    ind[:], iota[:].to_broadcast((128, TILE)), bc[:], op=iseq)
                g = gpp.tile([128, TILE], f32, tag="g")
                for (wi, a, b) in segs[t]:
                    nc.tensor.matmul(g[:, a:b], lhsT=zall[:, wi, :],
                                     rhs=ind[:, a:b], start=True, stop=True)
                bi, k = divmod(t, TPB)
                stg = stgs[bi % 2]
                nc.scalar.copy(
                    stg[:, k * TILE:(k + 1) * TILE, 0:1]
                    .rearrange("p e one -> p (e one)"), g[:])
                if taps and l == 0 and t < 4:
                    gt = smallp.tile([128, TILE], f32, tag="zo")
                    nc.vector.tensor_copy(gt[:], g[:])
                    nc.sync.dma_start(g_o.ap()[:, t * TILE:(t + 1) * TILE],
                                      gt[:])
                if k == TPB - 1:
                    if taps and l == 0:
                        nc.sync.dma_start(
                            stg_o.ap()[:, bi * BATCH * 2:(bi + 1) * BATCH * 2],
                            stg.rearrange("p e two -> p (e two)"))
                    view = agg[:, par:par + 2 * NELEMS].rearrange(
                        "p (e two) -> p e two", two=2)
                    nc.gpsimd.scatter_add(
                        view, idxt[:, bi * IPB:(bi + 1) * IPB], stg[:],
                        channels=128, num_elems=NELEMS, d=2, num_idxs=BATCH)

            # ---- GIN MLP (feature-major) --------------------------------
            if taps and l == 0:
                nc.sync.dma_start(agg_o.ap(), agg[:])
            h = zfm_nxt
            nc.vector.tensor_add(h[:], agg[:, 0:PER_CORE], zfm_cur[:])
            for s0 in range(0, PER_CORE, TILE):
                s1 = min(s0 + TILE, PER_CORE)
                sw = s1 - s0
                p1 = mlpp.tile([128, TILE], f32, tag="p1")
                nc.tensor.matmul(p1[:, 0:sw], lhsT=w1[:, l * 128:(l + 1) * 128],
                                 rhs=h[:, s0:s1], start=True, stop=True)
                h1 = smallp.tile([128, TILE], bf16, tag="h1")
                nc.scalar.activation(h1[:, 0:sw], p1[:, 0:sw], Relu,
                                     bias=b1[:, l:l + 1])
                p2 = mlpp.tile([128, TILE], f32, tag="p1")
                nc.tensor.matmul(p2[:, 0:sw], lhsT=w2[:, l * 128:(l + 1) * 128],
                                 rhs=h1[:, 0:sw], start=True, stop=True)
                if l < L - 1:
                    nc.scalar.activation(h[:, s0:s1], p2[:, 0:sw], Relu,
                                         bias=b2[:, l:l + 1])
                else:
                    zo = smallp.tile([128, TILE], f32, tag="zo")
                    nc.scalar.activation(zo[:, 0:sw], p2[:, 0:sw], Relu,
                                         bias=b2[:, l:l + 1])
                    nc.sync.dma_start(
                        zout_t.ap()[:, s0:s1], zo[:, 0:sw])

            if taps and l == 0:
                nc.sync.dma_start(z1_o.ap(), h[:])

            # ---- z_next -> node-major + halo ----------------------------
            if l < L - 1:
                zb3 = zblk[l].rearrange("(w p) d -> p w d", p=128)
                for g0 in range(0, NWC, 4):
                    gn = min(4, NWC - g0)
                    tp = tpp.tile([128, TILE], bf16, tag="tp")
                    for j in range(gn):
                        nc.tensor.transpose(
                            tp[:, j * 128:(j + 1) * 128],
                            h[:, (g0 + j) * 128:(g0 + j + 1) * 128],
                            ident[:])
                    zt = smallp.tile([128, TILE], bf16, tag="h1")
                    nc.scalar.copy(zt[:, 0:gn * 128], tp[:, 0:gn * 128])
                    nc.sync.dma_start(
                        zb3[:, g0:g0 + gn, :],
                        zt[:, 0:gn * 128].rearrange("p (w d) -> p w d", d=128))
                if collectives:
                    nc.gpsimd.collective_compute(
                        "AllGather", mybir.AluOpType.bypass,
                        replica_groups=rg,
                        ins=[zblk[l].opt()], outs=[zsh[l].opt()])
                else:
                    nc.sync.dma_start(
                        zsh[l].rearrange("(r n) d -> r n d", r=NCORES)[0],
                        zblk[l][:])
                nc.sync.dma_start(
                    zall[:], zsh[l].rearrange("(w p) d -> p w d", p=128))

    nc.compile()
    return nc


def _make_in_maps(inputs, geom, percore):
    import ml_dtypes
    bf = ml_dtypes.bfloat16
    x = np.asarray(inputs["x"], np.float32)
    Ws1 = np.asarray(inputs["Ws1"], np.float32)
    bs1 = np.asarray(inputs["bs1"], np.float32)
    Ws2 = np.asarray(inputs["Ws2"], np.float32)
    bs2 = np.asarray(inputs["bs2"], np.float32)

    xp = np.zeros((NPAD, D), np.float32)
    xp[:N] = x
    zall0 = np.ascontiguousarray(
        xp.reshape(NW, 128, D).transpose(1, 0, 2).reshape(128, NW * D)
    ).astype(bf)
    srcw_all, idxt_all, nblk = _pack_tables(geom, percore)
    emat = np.zeros((128, EBLK, 128), np.float32)
    for k in range(EBLK):
        emat[k, k, :] = 1.0
    emat = emat.reshape(128, EBLK * 128).astype(bf)
    iota = np.arange(128, dtype=np.float32).reshape(128, 1)
    ident = np.eye(128, dtype=np.float32).astype(bf)
    w1 = np.concatenate([Ws1[l] for l in range(L)], axis=1).astype(bf)
    w2 = np.concatenate([Ws2[l] for l in range(L)], axis=1).astype(bf)
    b1 = np.ascontiguousarray(bs1.T).astype(np.float32)
    b2 = np.ascontiguousarray(bs2.T).astype(np.float32)

    in_maps = []
    for c in range(NCORES):
        zfm0 = np.ascontiguousarray(
            xp[c * PER_CORE:(c + 1) * PER_CORE].T).astype(bf)
        in_maps.append({
            "zall0": zall0, "zfm0": zfm0,
            "srcw": srcw_all[c].astype(bf),
            "emat": emat, "iota": iota, "ident": ident,
            "idxt": idxt_all[c].astype(np.int16),
            "w1": w1, "w2": w2, "b1": b1, "b2": b2,
        })
    return in_maps


def kernel(x, Ws1, bs1, Ws2, bs2, edge_index):
    geom, percore = _prepare_edges(edge_index)
    in_maps = _make_in_maps(
        {"x": x, "Ws1": Ws1, "bs1": bs1, "Ws2": Ws2, "bs2": bs2},
        geom, percore)
    nc = _build_program(geom)

    from concourse.bass_utils import run_bass_kernel_spmd
    res = run_bass_kernel_spmd(nc, in_maps, core_ids=list(range(NCORES)))
    global last_results
    last_results = res

    out = np.empty((NPAD, D), np.float32)
    for c in range(NCORES):
        out[c * PER_CORE:(c + 1) * PER_CORE] = res.results[c]["zout"].T
    return out[:N]


if __name__ == "__main__":
    data = np.load("/root/problem/inputs.npz")
    geom, percore = _prepare_edges(data["edge_index"])
    print("TOTC:", geom["TOTC"], "ntiles:", geom["ntiles"],
          "nbatch:", geom["nbatch"],
          "inflation:", geom["TOTC"] / (E / NCORES))
    nseg = sum(len(s) for s in geom["segs"])
    print("total matmul segments per layer:", nseg)
    out = _numpy_sim({k: data[k] for k in data.files}, geom, percore)
    exp = np.load("/root/problem/expected.npy")
    err = np.abs(out - exp).max() / np.abs(exp).max()
    print("numpy-sim rel err:", err)


# revision 25
# speedup vs baseline: 2.8493x; 1.8565x over previous
"""GIN encoder (3-layer, N=50000, E=800000, D=128) on 8 trn2 NeuronCores.

v2 strategy — descriptor-free aggregation (no dma_gather):
  - Every core keeps the FULL node-feature table Z in SBUF, node-major
    bf16 [128 slots, 392 windows, 128 feat] (all-gathered per layer).
  - Edges partitioned by dst core; per core the edge stream is grouped
    into cells (parity(dst), src window), padded uniformly across cores
    (SPMD). Per 512-column tile:
      1. PE "broadcast" matmul (one-hot lhsT E_k) replicates the tile's
         per-edge src-slot values from a packed [128, *] table to all
         128 partitions (PSUM fp32).
      2. DVE is_equal vs a per-partition iota builds the slot indicator
         [slot, col] in bf16.
      3. One PE matmul per (window-run in tile) gathers z[src] columns:
         G[feat, col] = Z_win^T_slotmajor @ indicator  (PSUM fp32).
      4. ACT copies G into a staging ring, bf16, stride-2 (d=2 layout
         with a permanent-zero partner slot).
      5. gpsimd.scatter_add accumulates staging into the feature-major
         agg [128, npairs, 2] (bf16), idx = dst node-pair; the odd-dst
         pass uses a one-column-shifted view of the same agg buffer.
    scatter_add loses duplicate updates within an aligned 8-index octet
    (SIMD width 8), so same-pair edges are round-robined across octets
    per cell at prep time; pad columns add 0 to a dump pair.
  - MLP runs feature-major on [128, 6272] (h = agg + z), then the own
    z_next is PE-transposed to node-major, DMA'd to HBM and AllGathered
    for the next layer's Z table.
"""

import numpy as np

N = 50000
E = 800000
D = 128
L = 3
NCORES = 8
PER_CORE = 6272          # 49 * 128 dst nodes per core
NPAD = 50176             # 8 * 6272
NW = 392                 # global 128-node source windows
NWC = 49                 # windows per core
NPAIRS = 3136            # dst node pairs per core
TILE = 512               # column tile (one PSUM bank)
BATCH = 3584             # scatter_add batch = 7 tiles, %16 == 0
NELEMS = 3140            # scatter_add num_elems (3136 real + dump space)
DUMP = 3139              # dump pair for pad columns
PADV = 300.0             # src-slot value for pad columns (never matches)
EBLK = 32                # one-hot broadcast matrices E_0..E_63


def _prepare_edges(edge_index):
    """Build the uniform cell geometry + per-core tables.

    Returns (geom, percore) where geom has the shared static structure and
    percore the per-core srcvals/idx tables.
    """
    src = np.asarray(edge_index[0], dtype=np.int64)
    dst = np.asarray(edge_index[1], dtype=np.int64)

    core = dst // PER_CORE
    dloc = dst % PER_CORE
    par = dloc & 1
    w = src >> 7
    slot = src & 127
    pairv = np.where(par == 0, dloc >> 1, (dloc - 1) >> 1)

    # cell id per edge: (core, par, w)
    cell = (core * 2 + par) * NW + w
    ncells = NCORES * 2 * NW
    counts = np.bincount(cell, minlength=ncells)

    # max multiplicity of (cell, pair) — octet lower bound
    cp = cell * np.int64(NPAIRS) + pairv
    _, cp_counts = np.unique(cp, return_counts=True)
    cp_cell = np.unique(cp) // NPAIRS
    maxmult = np.zeros(ncells, np.int64)
    np.maximum.at(maxmult, cp_cell, cp_counts)

    # scatter_add loses duplicate-pair updates closer than ~4 idx columns
    # (two 8-lane units stream the columns with skew). Enforce SEP-position
    # separation between same-pair updates within a scatter batch.
    SEP = 96

    order = np.lexsort((pairv, cell))
    cell_s = cell[order]
    cell_starts = np.zeros(ncells + 1, np.int64)
    np.cumsum(counts, out=cell_starts[1:])

    # per-cell edge lists (sorted by pair) per core
    K = np.maximum((counts + 7) // 8, maxmult).reshape(NCORES, 2, NW).max(0)

    def place(K):
        """Greedy octet assignment honoring SEP. Returns (ok, needK,
        srcvals, idxvals, off, tot, base)."""
        P = K * 8
        off = np.zeros((2, NW), np.int64)
        tot = np.zeros(2, np.int64)
        for p in (0, 1):
            off[p] = np.cumsum(np.concatenate([[0], P[p][:-1]]))
            tot[p] = int(np.ceil(P[p].sum() / BATCH)) * BATCH
        base = np.array([0, tot[0]], np.int64)
        TOTC = int(tot.sum())
        srcvals = np.full((NCORES, TOTC), PADV, np.float64)
        idxvals = np.full((NCORES, TOTC), DUMP, np.int64)
        needK = K.copy()
        ok = True
        sepo = (SEP + 7) // 8
        for c in range(NCORES):
            for p in (0, 1):
                nextpos = {}
                for wi in range(NW):
                    cid = (c * 2 + p) * NW + wi
                    s0, s1 = cell_starts[cid], cell_starts[cid + 1]
                    if s0 == s1:
                        continue
                    kk = int(K[p, wi])
                    cbase = int(base[p] + off[p, wi])
                    cap = np.zeros(kk, np.int64)
                    fill = [[] for _ in range(kk)]
                    eidx = order[s0:s1]
                    prs = pairv[eidx]
                    # groups by pair, biggest first
                    upr, inv, cnt = np.unique(prs, return_inverse=True,
                                              return_counts=True)
                    gorder = np.argsort(-cnt)
                    failed = False
                    for gi in gorder:
                        members = eidx[inv == gi]
                        pr = int(upr[gi])
                        for e in members:
                            o0 = nextpos.get(pr, -10**9)
                            o0 = max(0, (o0 - cbase + 7) // 8)
                            o = o0
                            while o < kk and cap[o] >= 8:
                                o += 1
                            if o >= kk:
                                failed = True
                                needK[p, wi] = max(needK[p, wi],
                                                   kk + max(1, o0 - kk + 1))
                                continue
                            fill[o].append(e)
                            cap[o] += 1
                            nextpos[pr] = cbase + 8 * o + 8 + SEP
                    if failed:
                        ok = False
                        continue
                    for o in range(kk):
                        for li, e in enumerate(fill[o]):
                            col = cbase + 8 * o + li
                            srcvals[c, col] = slot[e]
                            idxvals[c, col] = pairv[e]
        return ok, needK, srcvals, idxvals, off, tot, base

    for _ in range(6):
        ok, needK, srcvals, idxvals, off, tot, base = place(K)
        if ok:
            break
        K = needK
    assert ok, "greedy octet placement failed"
    P = K * 8
    TOTC = int(tot.sum())
    assert TOTC % BATCH == 0
    ntiles = TOTC // TILE

    # verify: same-pair separation >= SEP within each batch, octets distinct
    for c in range(NCORES):
        for p in (0, 1):
            covs = np.arange(base[p], base[p] + tot[p])
            idb = idxvals[c, covs].reshape(-1, BATCH)
            for b in range(idb.shape[0]):
                row = idb[b]
                real = row != DUMP
                pos = np.arange(BATCH)[real]
                prs = row[real]
                o = np.lexsort((pos, prs))
                same = prs[o][1:] == prs[o][:-1]
                gap = pos[o][1:] - pos[o][:-1]
                assert not (same & (gap < SEP)).any(), "separation violated"

    # tile segments: per tile, runs of (w, a, b) in-tile col ranges
    # (uniform across cores). Pad ranges use window 0 (indicator all-zero).
    bounds = []              # (colstart, colend, w) in stream order
    for p in (0, 1):
        cstart = base[p]
        for wi in range(NW):
            if P[p, wi]:
                s0 = base[p] + off[p, wi]
                bounds.append((s0, s0 + P[p, wi], wi))
        pe = base[p] + P[p].sum()
        if tot[p] > P[p].sum():
            bounds.append((pe, base[p] + tot[p], 0))
    segs = [[] for _ in range(ntiles)]
    for (s0, s1, wi) in bounds:
        t0, t1 = s0 // TILE, (s1 - 1) // TILE
        for t in range(t0, t1 + 1):
            a = max(s0, t * TILE) - t * TILE
            b = min(s1, (t + 1) * TILE) - t * TILE
            segs[t].append((wi, int(a), int(b)))

    geom = {
        "TOTC": TOTC, "ntiles": ntiles, "segs": segs,
        "tot": tot, "base": base,
        "nbatch": TOTC // BATCH,
        "par_of_tile": [0 if t * TILE < tot[0] else 1 for t in range(ntiles)],
    }
    percore = {"srcvals": srcvals, "idxvals": idxvals}
    return geom, percore


def _pack_tables(geom, percore):
    """srcW packing + wrapped scatter idx tables, per core."""
    TOTC, ntiles = geom["TOTC"], geom["ntiles"]
    nblk = (ntiles + EBLK - 1) // EBLK
    srcw = np.zeros((NCORES, 128, nblk * TILE), np.float64)
    sv = percore["srcvals"].reshape(NCORES, ntiles, TILE)
    for t in range(ntiles):
        srcw[:, t % EBLK, (t // EBLK) * TILE:(t // EBLK + 1) * TILE] = sv[:, t]

    idx = percore["idxvals"].astype(np.int16)    # [NCORES, TOTC]
    nb = geom["nbatch"]
    iw = idx.reshape(NCORES, nb, BATCH // 16, 16)
    idxt = np.tile(iw.transpose(0, 3, 1, 2).reshape(NCORES, 16, nb * (BATCH // 16)),
                   (1, 8, 1))                    # [NCORES, 128, nb*224]
    return srcw, idxt, nblk


def _numpy_sim(inputs, geom, percore):
    """Bit-approximate pipeline sim (fp32 math) to validate the tables."""
    x = np.asarray(inputs["x"], np.float32)
    Ws1, bs1 = np.asarray(inputs["Ws1"], np.float32), np.asarray(inputs["bs1"], np.float32)
    Ws2, bs2 = np.asarray(inputs["Ws2"], np.float32), np.asarray(inputs["bs2"], np.float32)
    xp = np.zeros((NPAD, D), np.float32)
    xp[:N] = x
    z = xp.copy()
    sv = percore["srcvals"]
    iv = percore["idxvals"]
    tot, base = geom["tot"], geom["base"]
    for l in range(L):
        zn = np.zeros_like(z)
        for c in range(NCORES):
            agg2 = np.zeros((D, NELEMS + 1, 2), np.float32)
            # gather G columns
            segs = geom["segs"]
            G = np.zeros((D, geom["TOTC"]), np.float32)
            for t, seglist in enumerate(segs):
                for (wi, a, b) in seglist:
                    cols = np.arange(t * TILE + a, t * TILE + b)
                    s = sv[c, cols]
                    real = s < 128
                    gsl = np.zeros((D, len(cols)), np.float32)
                    nodes = wi * 128 + s[real].astype(np.int64)
                    gsl[:, real] = z[nodes].T
                    G[:, cols] = gsl
            # scatter (true accumulation; octet constraint already asserted)
            for p in (0, 1):
                cols = np.arange(base[p], base[p] + tot[p])
                idxs = iv[c, cols]
                tgt = np.zeros((NELEMS + 1, D), np.float32)
                np.add.at(tgt, idxs, G[:, cols].T)
                agg2[:, :, p] += tgt.T
            # unpack agg2 -> agg cols: even pass wrote (pair k -> col 2k),
            # odd pass wrote (pair k -> col 2k+1)
            agg = np.zeros((D, PER_CORE), np.float32)
            agg[:, 0::2] = agg2[:, :NPAIRS, 0]
            agg[:, 1::2] = agg2[:, :NPAIRS, 1]
            zc = z[c * PER_CORE:(c + 1) * PER_CORE].T
            h = agg + zc
            h1 = np.maximum(Ws1[l].T @ h + bs1[l][:, None], 0)
            z2 = np.maximum(Ws2[l].T @ h1 + bs2[l][:, None], 0)
            zn[c * PER_CORE:(c + 1) * PER_CORE] = z2.T
        z = zn
    return z[:N]


def _build_program(geom, n_devices=NCORES, collectives=True, taps=False):
    import concourse.bacc as bacc
    import concourse.tile as tile
    import concourse.mybir as mybir
    from contextlib import ExitStack

    f32 = mybir.dt.float32
    bf16 = mybir.dt.bfloat16
    i16 = mybir.dt.int16
    Relu = mybir.ActivationFunctionType.Relu
    iseq = mybir.AluOpType.is_equal

    ntiles = geom["ntiles"]
    segs = geom["segs"]
    nb = geom["nbatch"]
    nblk = (ntiles + EBLK - 1) // EBLK
    TPB = BATCH // TILE          # tiles per scatter batch (7)
    IPB = BATCH // 16            # idx cols per batch (224)

    nc = bacc.Bacc("TRN2", debug=False, enable_asserts=False,
                   target_bir_lowering=False, num_devices=n_devices)

    zall0_t = nc.dram_tensor("zall0", [128, NW * 128], bf16, kind="ExternalInput")
    zfm0_t = nc.dram_tensor("zfm0", [128, PER_CORE], bf16, kind="ExternalInput")
    srcw_t = nc.dram_tensor("srcw", [128, nblk * TILE], bf16, kind="ExternalInput")
    emat_t = nc.dram_tensor("emat", [128, EBLK * 128], bf16, kind="ExternalInput")
    iota_t = nc.dram_tensor("iota", [128, 1], f32, kind="ExternalInput")
    ident_t = nc.dram_tensor("ident", [128, 128], bf16, kind="ExternalInput")
    idxt_t = nc.dram_tensor("idxt", [128, nb * IPB], i16, kind="ExternalInput")
    w1_t = nc.dram_tensor("w1", [128, L * 128], bf16, kind="ExternalInput")
    w2_t = nc.dram_tensor("w2", [128, L * 128], bf16, kind="ExternalInput")
    b1_t = nc.dram_tensor("b1", [128, L], f32, kind="ExternalInput")
    b2_t = nc.dram_tensor("b2", [128, L], f32, kind="ExternalInput")
    zout_t = nc.dram_tensor("zout", [128, PER_CORE], f32, kind="ExternalOutput")
    if taps:
        agg_o = nc.dram_tensor("agg_o", [128, 2 * NELEMS + 1], bf16,
                               kind="ExternalOutput")
        z1_o = nc.dram_tensor("z1_o", [128, PER_CORE], bf16,
                              kind="ExternalOutput")
        g_o = nc.dram_tensor("g_o", [128, 4 * TILE], f32,
                             kind="ExternalOutput")
        stg_o = nc.dram_tensor("stg_o", [128, geom["nbatch"] * BATCH * 2],
                               bf16, kind="ExternalOutput")

    rg = [list(range(NCORES))]

    with tile.TileContext(nc) as tc, ExitStack() as ctx:
        const = ctx.enter_context(tc.tile_pool(name="const", bufs=1))
        zap = ctx.enter_context(tc.tile_pool(name="za", bufs=1))
        zfp = ctx.enter_context(tc.tile_pool(name="zf", bufs=1))
        agp = ctx.enter_context(tc.tile_pool(name="ag", bufs=1))
        stp = ctx.enter_context(tc.tile_pool(name="st", bufs=1))
        indp = ctx.enter_context(tc.tile_pool(name="ind", bufs=2))
        smallp = ctx.enter_context(tc.tile_pool(name="sm", bufs=2))
        bcp = ctx.enter_context(tc.tile_pool(name="bc", bufs=2, space="PSUM"))
        gpp = ctx.enter_context(tc.tile_pool(name="gp", bufs=2, space="PSUM"))
        mlpp = ctx.enter_context(tc.tile_pool(name="mlp", bufs=2, space="PSUM"))
        tpp = ctx.enter_context(tc.tile_pool(name="tp", bufs=2, space="PSUM"))
        dram = ctx.enter_context(tc.tile_pool(name="dram", bufs=1, space="DRAM"))

        srcw = const.tile([128, nblk * TILE], bf16)
        emat = const.tile([128, EBLK * 128], bf16)
        iota = const.tile([128, 1], f32)
        ident = const.tile([128, 128], bf16)
        idxt = const.tile([128, nb * IPB], i16)
        w1 = const.tile([128, L * 128], bf16)
        w2 = const.tile([128, L * 128], bf16)
        b1 = const.tile([128, L], f32)
        b2 = const.tile([128, L], f32)
        for sb, t in ((srcw, srcw_t), (emat, emat_t), (iota, iota_t),
                      (ident, ident_t), (idxt, idxt_t), (w1, w1_t),
                      (w2, w2_t), (b1, b1_t), (b2, b2_t)):
            nc.sync.dma_start(sb[:], t.ap())

        zall = zap.tile([128, NW, 128], bf16)
        nc.sync.dma_start(zall.rearrange("p w d -> p (w d)"), zall0_t.ap())
        zfmA = zfp.tile([128, PER_CORE], bf16)
        zfmB = zfp.tile([128, PER_CORE], bf16)
        nc.sync.dma_start(zfmA[:], zfm0_t.ap())
        agg = agp.tile([128, 2 * NELEMS + 1], bf16)
        stgs = [stp.tile([128, BATCH, 2], bf16, name=f"stg{i}") for i in (0, 1)]
        for s in stgs:
            nc.vector.memset(s.rearrange("p e two -> p (e two)"), 0.0)

        # node-major halo blocks: [128 slot-partitions, PER_CORE] per core;
        # AllGather concatenates along dim 0 -> [8*128, PER_CORE]
        zblk = [dram.tile([128, PER_CORE], bf16, name=f"zblk{l}", tag=f"zblk{l}")
                for l in range(L - 1)]
        sh = "Shared" if collectives else "Local"
        zsh = [dram.tile([NCORES * 128, PER_CORE], bf16, addr_space=sh,
                         name=f"zsh{l}", tag=f"zsh{l}") for l in range(L - 1)]

        for l in range(L):
            zfm_cur = zfmA if l % 2 == 0 else zfmB
            zfm_nxt = zfmB if l % 2 == 0 else zfmA
            nc.vector.memset(agg[:], 0.0)

            for t in range(ntiles):
                par = geom["par_of_tile"][t]
                bc = bcp.tile([128, TILE], f32, tag="bc")
                nc.tensor.matmul(
                    bc[:], lhsT=emat[:, (t % EBLK) * 128:(t % EBLK + 1) * 128],
                    rhs=srcw[:, (t // EBLK) * TILE:(t // EBLK + 1) * TILE],
                    start=True, stop=True)
                ind = indp.tile([128, TILE], bf16, tag="ind")
                nc.vector.tensor_tensor(
                    ind[:], iota[:].to_broadcast((128, TILE)), bc[:], op=iseq)
                g = gpp.tile([128, TILE], f32, tag="g")
                for (wi, a, b) in segs[t]:
                    nc.tensor.matmul(g[:, a:b], lhsT=zall[:, wi, :],
                                     rhs=ind[:, a:b], start=True, stop=True)
                bi, k = divmod(t, TPB)
                stg = stgs[bi % 2]
                nc.scalar.copy(
                    stg[:, k * TILE:(k + 1) * TILE, 0:1]
                    .rearrange("p e one -> p (e one)"), g[:])
                if taps and l == 0 and t < 4:
                    gt = smallp.tile([128, TILE], f32, tag="zo")
                    nc.vector.tensor_copy(gt[:], g[:])
                    nc.sync.dma_start(g_o.ap()[:, t * TILE:(t + 1) * TILE],
                                      gt[:])
                if k == TPB - 1:
                    if taps and l == 0:
                        nc.sync.dma_start(
                            stg_o.ap()[:, bi * BATCH * 2:(bi + 1) * BATCH * 2],
                            stg.rearrange("p e two -> p (e two)"))
                    view = agg[:, par:par + 2 * NELEMS].rearrange(
                        "p (e two) -> p e two", two=2)
                    nc.gpsimd.scatter_add(
                        view, idxt[:, bi * IPB:(bi + 1) * IPB], stg[:],
                        channels=128, num_elems=NELEMS, d=2, num_idxs=BATCH)

            # ---- GIN MLP (feature-major) --------------------------------
            if taps and l == 0:
                nc.sync.dma_start(agg_o.ap(), agg[:])
            h = zfm_nxt
            nc.vector.tensor_add(h[:], agg[:, 0:PER_CORE], zfm_cur[:])
            for s0 in range(0, PER_CORE, TILE):
                s1 = min(s0 + TILE, PER_CORE)
                sw = s1 - s0
                p1 = mlpp.tile([128, TILE], f32, tag="p1")
                nc.tensor.matmul(p1[:, 0:sw], lhsT=w1[:, l * 128:(l + 1) * 128],
                                 rhs=h[:, s0:s1], start=True, stop=True)
                h1 = smallp.tile([128, TILE], bf16, tag="h1")
                nc.scalar.activation(h1[:, 0:sw], p1[:, 0:sw], Relu,
                                     bias=b1[:, l:l + 1])
                p2 = mlpp.tile([128, TILE], f32, tag="p1")
                nc.tensor.matmul(p2[:, 0:sw], lhsT=w2[:, l * 128:(l + 1) * 128],
                                 rhs=h1[:, 0:sw], start=True, stop=True)
                if l < L - 1:
                    nc.scalar.activation(h[:, s0:s1], p2[:, 0:sw], Relu,
                                         bias=b2[:, l:l + 1])
                else:
                    zo = smallp.tile([128, TILE], f32, tag="zo")
                    nc.scalar.activation(zo[:, 0:sw], p2[:, 0:sw], Relu,
                                         bias=b2[:, l:l + 1])
                    nc.sync.dma_start(
                        zout_t.ap()[:, s0:s1], zo[:, 0:sw])

            if taps and l == 0:
                nc.sync.dma_start(z1_o.ap(), h[:])

            # ---- z_next -> node-major + halo ----------------------------
            if l < L - 1:
                for g0 in range(0, NWC, 4):
                    gn = min(4, NWC - g0)
                    tp = tpp.tile([128, TILE], bf16, tag="tp")
                    for j in range(gn):
                        nc.tensor.transpose(
                            tp[:, j * 128:(j + 1) * 128],
                            h[:, (g0 + j) * 128:(g0 + j + 1) * 128],
                            ident[:])
                    zt = smallp.tile([128, TILE], bf16, tag="h1")
                    nc.scalar.copy(zt[:, 0:gn * 128], tp[:, 0:gn * 128])
                    nc.sync.dma_start(
                        zblk[l][:, g0 * 128:(g0 + gn) * 128],
                        zt[:, 0:gn * 128])
                if collectives:
                    nc.gpsimd.collective_compute(
                        "AllGather", mybir.AluOpType.bypass,
                        replica_groups=rg,
                        ins=[zblk[l].opt()], outs=[zsh[l].opt()])
                else:
                    nc.sync.dma_start(
                        zsh[l].rearrange("(r p) n -> r p n", r=NCORES)[0],
                        zblk[l][:])
                zaf = zall.rearrange("p w d -> p (w d)")
                for r in range(NCORES):
                    nc.sync.dma_start(
                        zaf[:, r * PER_CORE:(r + 1) * PER_CORE],
                        zsh[l][r * 128:(r + 1) * 128, :])

    nc.compile()
    return nc


def _make_in_maps(inputs, geom, percore):
    import ml_dtypes
    bf = ml_dtypes.bfloat16
    x = np.asarray(inputs["x"], np.float32)
    Ws1 = np.asarray(inputs["Ws1"], np.float32)
    bs1 = np.asarray(inputs["bs1"], np.float32)
    Ws2 = np.asarray(inputs["Ws2"], np.float32)
    bs2 = np.asarray(inputs["bs2"], np.float32)

    xp = np.zeros((NPAD, D), np.float32)
    xp[:N] = x
    zall0 = np.ascontiguousarray(
        xp.reshape(NW, 128, D).transpose(1, 0, 2).reshape(128, NW * D)
    ).astype(bf)
    srcw_all, idxt_all, nblk = _pack_tables(geom, percore)
    emat = np.zeros((128, EBLK, 128), np.float32)
    for k in range(EBLK):
        emat[k, k, :] = 1.0
    emat = emat.reshape(128, EBLK * 128).astype(bf)
    iota = np.arange(128, dtype=np.float32).reshape(128, 1)
    ident = np.eye(128, dtype=np.float32).astype(bf)
    w1 = np.concatenate([Ws1[l] for l in range(L)], axis=1).astype(bf)
    w2 = np.concatenate([Ws2[l] for l in range(L)], axis=1).astype(bf)
    b1 = np.ascontiguousarray(bs1.T).astype(np.float32)
    b2 = np.ascontiguousarray(bs2.T).astype(np.float32)

    in_maps = []
    for c in range(NCORES):
        zfm0 = np.ascontiguousarray(
            xp[c * PER_CORE:(c + 1) * PER_CORE].T).astype(bf)
        in_maps.append({
            "zall0": zall0, "zfm0": zfm0,
            "srcw": srcw_all[c].astype(bf),
            "emat": emat, "iota": iota, "ident": ident,
            "idxt": idxt_all[c].astype(np.int16),
            "w1": w1, "w2": w2, "b1": b1, "b2": b2,
        })
    return in_maps


def kernel(x, Ws1, bs1, Ws2, bs2, edge_index):
    geom, percore = _prepare_edges(edge_index)
    in_maps = _make_in_maps(
        {"x": x, "Ws1": Ws1, "bs1": bs1, "Ws2": Ws2, "bs2": bs2},
        geom, percore)
    nc = _build_program(geom)

    from concourse.bass_utils import run_bass_kernel_spmd
    res = run_bass_kernel_spmd(nc, in_maps, core_ids=list(range(NCORES)))
    global last_results
    last_results = res

    out = np.empty((NPAD, D), np.float32)
    for c in range(NCORES):
        out[c * PER_CORE:(c + 1) * PER_CORE] = res.results[c]["zout"].T
    return out[:N]


if __name__ == "__main__":
    data = np.load("/root/problem/inputs.npz")
    geom, percore = _prepare_edges(data["edge_index"])
    print("TOTC:", geom["TOTC"], "ntiles:", geom["ntiles"],
          "nbatch:", geom["nbatch"],
          "inflation:", geom["TOTC"] / (E / NCORES))
    nseg = sum(len(s) for s in geom["segs"])
    print("total matmul segments per layer:", nseg)
    out = _numpy_sim({k: data[k] for k in data.files}, geom, percore)
    exp = np.load("/root/problem/expected.npy")
    err = np.abs(out - exp).max() / np.abs(exp).max()
    print("numpy-sim rel err:", err)


# revision 28
# speedup vs baseline: 2.9116x; 1.0219x over previous
"""GIN encoder (3-layer, N=50000, E=800000, D=128) on 8 trn2 NeuronCores.

v2 strategy — descriptor-free aggregation (no dma_gather):
  - Every core keeps the FULL node-feature table Z in SBUF, node-major
    bf16 [128 slots, 392 windows, 128 feat] (all-gathered per layer).
  - Edges partitioned by dst core; per core the edge stream is grouped
    into cells (parity(dst), src window), padded uniformly across cores
    (SPMD). Per 512-column tile:
      1. PE "broadcast" matmul (one-hot lhsT E_k) replicates the tile's
         per-edge src-slot values from a packed [128, *] table to all
         128 partitions (PSUM fp32).
      2. DVE is_equal vs a per-partition iota builds the slot indicator
         [slot, col] in bf16.
      3. One PE matmul per (window-run in tile) gathers z[src] columns:
         G[feat, col] = Z_win^T_slotmajor @ indicator  (PSUM fp32).
      4. ACT copies G into a staging ring, bf16, stride-2 (d=2 layout
         with a permanent-zero partner slot).
      5. gpsimd.scatter_add accumulates staging into the feature-major
         agg [128, npairs, 2] (bf16), idx = dst node-pair; the odd-dst
         pass uses a one-column-shifted view of the same agg buffer.
    scatter_add loses duplicate updates within an aligned 8-index octet
    (SIMD width 8), so same-pair edges are round-robined across octets
    per cell at prep time; pad columns add 0 to a dump pair.
  - MLP runs feature-major on [128, 6272] (h = agg + z), then the own
    z_next is PE-transposed to node-major, DMA'd to HBM and AllGathered
    for the next layer's Z table.
"""

import numpy as np

N = 50000
E = 800000
D = 128
L = 3
NCORES = 8
PER_CORE = 6272          # 49 * 128 dst nodes per core
NPAD = 50176             # 8 * 6272
NW = 392                 # global 128-node source windows
NWC = 49                 # windows per core
NPAIRS = 3136            # dst node pairs per core
TILE = 512               # column tile (one PSUM bank)
BATCH = 3584             # scatter_add batch = 7 tiles, %16 == 0
NELEMS = 3140            # scatter_add num_elems (3136 real + dump space)
DUMP = 3139              # dump pair for pad columns
PADV = 300.0             # src-slot value for pad columns (never matches)
EBLK = 32                # one-hot broadcast matrices E_0..E_63


def _prepare_edges(edge_index):
    """Build the uniform cell geometry + per-core tables.

    Returns (geom, percore) where geom has the shared static structure and
    percore the per-core srcvals/idx tables.
    """
    src = np.asarray(edge_index[0], dtype=np.int64)
    dst = np.asarray(edge_index[1], dtype=np.int64)

    core = dst // PER_CORE
    dloc = dst % PER_CORE
    par = dloc & 1
    w = src >> 7
    slot = src & 127
    pairv = np.where(par == 0, dloc >> 1, (dloc - 1) >> 1)

    # cell id per edge: (core, par, w)
    cell = (core * 2 + par) * NW + w
    ncells = NCORES * 2 * NW
    counts = np.bincount(cell, minlength=ncells)

    # max multiplicity of (cell, pair) — octet lower bound
    cp = cell * np.int64(NPAIRS) + pairv
    _, cp_counts = np.unique(cp, return_counts=True)
    cp_cell = np.unique(cp) // NPAIRS
    maxmult = np.zeros(ncells, np.int64)
    np.maximum.at(maxmult, cp_cell, cp_counts)

    # scatter_add loses duplicate-pair updates closer than ~4 idx columns
    # (two 8-lane units stream the columns with skew). Enforce SEP-position
    # separation between same-pair updates within a scatter batch.
    SEP = 96

    order = np.lexsort((pairv, cell))
    cell_s = cell[order]
    cell_starts = np.zeros(ncells + 1, np.int64)
    np.cumsum(counts, out=cell_starts[1:])

    # per-cell edge lists (sorted by pair) per core
    K = np.maximum((counts + 7) // 8, maxmult).reshape(NCORES, 2, NW).max(0)

    def place(K):
        """Greedy octet assignment honoring SEP. Returns (ok, needK,
        srcvals, idxvals, off, tot, base)."""
        P = K * 8
        off = np.zeros((2, NW), np.int64)
        tot = np.zeros(2, np.int64)
        for p in (0, 1):
            off[p] = np.cumsum(np.concatenate([[0], P[p][:-1]]))
            tot[p] = int(np.ceil(P[p].sum() / BATCH)) * BATCH
        base = np.array([0, tot[0]], np.int64)
        TOTC = int(tot.sum())
        srcvals = np.full((NCORES, TOTC), PADV, np.float64)
        idxvals = np.full((NCORES, TOTC), DUMP, np.int64)
        needK = K.copy()
        ok = True
        sepo = (SEP + 7) // 8
        for c in range(NCORES):
            for p in (0, 1):
                nextpos = {}
                for wi in range(NW):
                    cid = (c * 2 + p) * NW + wi
                    s0, s1 = cell_starts[cid], cell_starts[cid + 1]
                    if s0 == s1:
                        continue
                    kk = int(K[p, wi])
                    cbase = int(base[p] + off[p, wi])
                    cap = np.zeros(kk, np.int64)
                    fill = [[] for _ in range(kk)]
                    eidx = order[s0:s1]
                    prs = pairv[eidx]
                    # groups by pair, biggest first
                    upr, inv, cnt = np.unique(prs, return_inverse=True,
                                              return_counts=True)
                    gorder = np.argsort(-cnt)
                    failed = False
                    for gi in gorder:
                        members = eidx[inv == gi]
                        pr = int(upr[gi])
                        for e in members:
                            o0 = nextpos.get(pr, -10**9)
                            o0 = max(0, (o0 - cbase + 7) // 8)
                            o = o0
                            while o < kk and cap[o] >= 8:
                                o += 1
                            if o >= kk:
                                failed = True
                                needK[p, wi] = max(needK[p, wi],
                                                   kk + max(1, o0 - kk + 1))
                                continue
                            fill[o].append(e)
                            cap[o] += 1
                            nextpos[pr] = cbase + 8 * o + 8 + SEP
                    if failed:
                        ok = False
                        continue
                    for o in range(kk):
                        for li, e in enumerate(fill[o]):
                            col = cbase + 8 * o + li
                            srcvals[c, col] = slot[e]
                            idxvals[c, col] = pairv[e]
        return ok, needK, srcvals, idxvals, off, tot, base

    for _ in range(6):
        ok, needK, srcvals, idxvals, off, tot, base = place(K)
        if ok:
            break
        K = needK
    assert ok, "greedy octet placement failed"
    P = K * 8
    TOTC = int(tot.sum())
    assert TOTC % BATCH == 0
    ntiles = TOTC // TILE

    # verify: same-pair separation >= SEP within each batch, octets distinct
    for c in range(NCORES):
        for p in (0, 1):
            covs = np.arange(base[p], base[p] + tot[p])
            idb = idxvals[c, covs].reshape(-1, BATCH)
            for b in range(idb.shape[0]):
                row = idb[b]
                real = row != DUMP
                pos = np.arange(BATCH)[real]
                prs = row[real]
                o = np.lexsort((pos, prs))
                same = prs[o][1:] == prs[o][:-1]
                gap = pos[o][1:] - pos[o][:-1]
                assert not (same & (gap < SEP)).any(), "separation violated"

    # tile segments: per tile, runs of (w, a, b) in-tile col ranges
    # (uniform across cores). Pad ranges use window 0 (indicator all-zero).
    bounds = []              # (colstart, colend, w) in stream order
    for p in (0, 1):
        cstart = base[p]
        for wi in range(NW):
            if P[p, wi]:
                s0 = base[p] + off[p, wi]
                bounds.append((s0, s0 + P[p, wi], wi))
        pe = base[p] + P[p].sum()
        if tot[p] > P[p].sum():
            bounds.append((pe, base[p] + tot[p], 0))
    segs = [[] for _ in range(ntiles)]
    for (s0, s1, wi) in bounds:
        t0, t1 = s0 // TILE, (s1 - 1) // TILE
        for t in range(t0, t1 + 1):
            a = max(s0, t * TILE) - t * TILE
            b = min(s1, (t + 1) * TILE) - t * TILE
            segs[t].append((wi, int(a), int(b)))

    geom = {
        "TOTC": TOTC, "ntiles": ntiles, "segs": segs,
        "tot": tot, "base": base,
        "nbatch": TOTC // BATCH,
        "par_of_tile": [0 if t * TILE < tot[0] else 1 for t in range(ntiles)],
    }
    percore = {"srcvals": srcvals, "idxvals": idxvals}
    return geom, percore


def _pack_tables(geom, percore):
    """srcW packing + wrapped scatter idx tables, per core."""
    TOTC, ntiles = geom["TOTC"], geom["ntiles"]
    nblk = (ntiles + EBLK - 1) // EBLK
    srcw = np.zeros((NCORES, 128, nblk * TILE), np.float64)
    sv = percore["srcvals"].reshape(NCORES, ntiles, TILE)
    for t in range(ntiles):
        srcw[:, t % EBLK, (t // EBLK) * TILE:(t // EBLK + 1) * TILE] = sv[:, t]

    idx = percore["idxvals"].astype(np.int16)    # [NCORES, TOTC]
    nb = geom["nbatch"]
    iw = idx.reshape(NCORES, nb, BATCH // 16, 16)
    idxt = np.tile(iw.transpose(0, 3, 1, 2).reshape(NCORES, 16, nb * (BATCH // 16)),
                   (1, 8, 1))                    # [NCORES, 128, nb*224]
    return srcw, idxt, nblk


def _numpy_sim(inputs, geom, percore):
    """Bit-approximate pipeline sim (fp32 math) to validate the tables."""
    x = np.asarray(inputs["x"], np.float32)
    Ws1, bs1 = np.asarray(inputs["Ws1"], np.float32), np.asarray(inputs["bs1"], np.float32)
    Ws2, bs2 = np.asarray(inputs["Ws2"], np.float32), np.asarray(inputs["bs2"], np.float32)
    xp = np.zeros((NPAD, D), np.float32)
    xp[:N] = x
    z = xp.copy()
    sv = percore["srcvals"]
    iv = percore["idxvals"]
    tot, base = geom["tot"], geom["base"]
    for l in range(L):
        zn = np.zeros_like(z)
        for c in range(NCORES):
            agg2 = np.zeros((D, NELEMS + 1, 2), np.float32)
            # gather G columns
            segs = geom["segs"]
            G = np.zeros((D, geom["TOTC"]), np.float32)
            for t, seglist in enumerate(segs):
                for (wi, a, b) in seglist:
                    cols = np.arange(t * TILE + a, t * TILE + b)
                    s = sv[c, cols]
                    real = s < 128
                    gsl = np.zeros((D, len(cols)), np.float32)
                    nodes = wi * 128 + s[real].astype(np.int64)
                    gsl[:, real] = z[nodes].T
                    G[:, cols] = gsl
            # scatter (true accumulation; octet constraint already asserted)
            for p in (0, 1):
                cols = np.arange(base[p], base[p] + tot[p])
                idxs = iv[c, cols]
                tgt = np.zeros((NELEMS + 1, D), np.float32)
                np.add.at(tgt, idxs, G[:, cols].T)
                agg2[:, :, p] += tgt.T
            # unpack agg2 -> agg cols: even pass wrote (pair k -> col 2k),
            # odd pass wrote (pair k -> col 2k+1)
            agg = np.zeros((D, PER_CORE), np.float32)
            agg[:, 0::2] = agg2[:, :NPAIRS, 0]
            agg[:, 1::2] = agg2[:, :NPAIRS, 1]
            zc = z[c * PER_CORE:(c + 1) * PER_CORE].T
            h = agg + zc
            h1 = np.maximum(Ws1[l].T @ h + bs1[l][:, None], 0)
            z2 = np.maximum(Ws2[l].T @ h1 + bs2[l][:, None], 0)
            zn[c * PER_CORE:(c + 1) * PER_CORE] = z2.T
        z = zn
    return z[:N]


def _build_program(geom, n_devices=NCORES, collectives=True, taps=False):
    import concourse.bacc as bacc
    import concourse.tile as tile
    import concourse.mybir as mybir
    from contextlib import ExitStack

    f32 = mybir.dt.float32
    bf16 = mybir.dt.bfloat16
    i16 = mybir.dt.int16
    Relu = mybir.ActivationFunctionType.Relu
    iseq = mybir.AluOpType.is_equal

    ntiles = geom["ntiles"]
    segs = geom["segs"]
    nb = geom["nbatch"]
    nblk = (ntiles + EBLK - 1) // EBLK
    TPB = BATCH // TILE          # tiles per scatter batch (7)
    IPB = BATCH // 16            # idx cols per batch (224)

    nc = bacc.Bacc("TRN2", debug=False, enable_asserts=False,
                   target_bir_lowering=False, num_devices=n_devices)

    zall0_t = nc.dram_tensor("zall0", [128, NW * 128], bf16, kind="ExternalInput")
    zfm0_t = nc.dram_tensor("zfm0", [128, PER_CORE], bf16, kind="ExternalInput")
    srcw_t = nc.dram_tensor("srcw", [128, nblk * TILE], bf16, kind="ExternalInput")
    emat_t = nc.dram_tensor("emat", [128, EBLK * 128], bf16, kind="ExternalInput")
    iota_t = nc.dram_tensor("iota", [128, 1], f32, kind="ExternalInput")
    ident_t = nc.dram_tensor("ident", [128, 128], bf16, kind="ExternalInput")
    idxt_t = nc.dram_tensor("idxt", [128, nb * IPB], i16, kind="ExternalInput")
    w1_t = nc.dram_tensor("w1", [128, L * 128], bf16, kind="ExternalInput")
    w2_t = nc.dram_tensor("w2", [128, L * 128], bf16, kind="ExternalInput")
    b1_t = nc.dram_tensor("b1", [128, L], f32, kind="ExternalInput")
    b2_t = nc.dram_tensor("b2", [128, L], f32, kind="ExternalInput")
    zout_t = nc.dram_tensor("zout", [128, PER_CORE], f32, kind="ExternalOutput")
    if taps:
        agg_o = nc.dram_tensor("agg_o", [128, 2 * NELEMS + 1], bf16,
                               kind="ExternalOutput")
        z1_o = nc.dram_tensor("z1_o", [128, PER_CORE], bf16,
                              kind="ExternalOutput")
        g_o = nc.dram_tensor("g_o", [128, 4 * TILE], f32,
                             kind="ExternalOutput")
        stg_o = nc.dram_tensor("stg_o", [128, geom["nbatch"] * BATCH * 2],
                               bf16, kind="ExternalOutput")

    rg = [list(range(NCORES))]

    with tile.TileContext(nc) as tc, ExitStack() as ctx:
        const = ctx.enter_context(tc.tile_pool(name="const", bufs=1))
        zap = ctx.enter_context(tc.tile_pool(name="za", bufs=1))
        zfp = ctx.enter_context(tc.tile_pool(name="zf", bufs=1))
        agp = ctx.enter_context(tc.tile_pool(name="ag", bufs=1))
        stp = ctx.enter_context(tc.tile_pool(name="st", bufs=1))
        indp = ctx.enter_context(tc.tile_pool(name="ind", bufs=2))
        smallp = ctx.enter_context(tc.tile_pool(name="sm", bufs=2))
        bcp = ctx.enter_context(tc.tile_pool(name="bc", bufs=2, space="PSUM"))
        gpp = ctx.enter_context(tc.tile_pool(name="gp", bufs=2, space="PSUM"))
        mlpp = ctx.enter_context(tc.tile_pool(name="mlp", bufs=2, space="PSUM"))
        tpp = ctx.enter_context(tc.tile_pool(name="tp", bufs=2, space="PSUM"))
        dram = ctx.enter_context(tc.tile_pool(name="dram", bufs=1, space="DRAM"))

        srcw = const.tile([128, nblk * TILE], bf16)
        emat = const.tile([128, EBLK * 128], bf16)
        iota = const.tile([128, 1], f32)
        ident = const.tile([128, 128], bf16)
        idxt = const.tile([128, nb * IPB], i16)
        w1 = const.tile([128, L * 128], bf16)
        w2 = const.tile([128, L * 128], bf16)
        b1 = const.tile([128, L], f32)
        b2 = const.tile([128, L], f32)
        for sb, t in ((srcw, srcw_t), (emat, emat_t), (iota, iota_t),
                      (ident, ident_t), (idxt, idxt_t), (w1, w1_t),
                      (w2, w2_t), (b1, b1_t), (b2, b2_t)):
            nc.sync.dma_start(sb[:], t.ap())

        zall = [zap.tile([128, NWC, 128], bf16, name=f"zall{r}")
                for r in range(NCORES)]
        for r in range(NCORES):
            nc.sync.dma_start(
                zall[r].rearrange("p w d -> p (w d)"),
                zall0_t.ap()[:, r * PER_CORE:(r + 1) * PER_CORE])
        zfmA = zfp.tile([128, PER_CORE], bf16)
        zfmB = zfp.tile([128, PER_CORE], bf16)
        nc.sync.dma_start(zfmA[:], zfm0_t.ap())
        agg = agp.tile([128, 2 * NELEMS + 1], bf16)
        stgs = [stp.tile([128, BATCH, 2], bf16, name=f"stg{i}") for i in (0, 1)]
        for s in stgs:
            nc.vector.memset(s.rearrange("p e two -> p (e two)"), 0.0)

        # node-major halo blocks: [128 slot-partitions, PER_CORE] per core;
        # AllGather concatenates along dim 0 -> [8*128, PER_CORE]
        zblk = [dram.tile([128, PER_CORE], bf16, name=f"zblk{l}", tag=f"zblk{l}")
                for l in range(L - 1)]
        sh = "Shared" if collectives else "Local"
        zsh = [dram.tile([NCORES * 128, PER_CORE], bf16, addr_space=sh,
                         name=f"zsh{l}", tag=f"zsh{l}") for l in range(L - 1)]

        for l in range(L):
            zfm_cur = zfmA if l % 2 == 0 else zfmB
            zfm_nxt = zfmB if l % 2 == 0 else zfmA
            nc.vector.memset(agg[:], 0.0)

            for t in range(ntiles):
                par = geom["par_of_tile"][t]
                bc = bcp.tile([128, TILE], f32, tag="bc")
                nc.tensor.matmul(
                    bc[:], lhsT=emat[:, (t % EBLK) * 128:(t % EBLK + 1) * 128],
                    rhs=srcw[:, (t // EBLK) * TILE:(t // EBLK + 1) * TILE],
                    start=True, stop=True)
                ind = indp.tile([128, TILE], bf16, tag="ind")
                nc.vector.tensor_tensor(
                    ind[:], iota[:].to_broadcast((128, TILE)), bc[:], op=iseq)
                g = gpp.tile([128, TILE], f32, tag="g")
                for (wi, a, b) in segs[t]:
                    nc.tensor.matmul(g[:, a:b],
                                     lhsT=zall[wi // NWC][:, wi % NWC, :],
                                     rhs=ind[:, a:b], start=True, stop=True)
                bi, k = divmod(t, TPB)
                stg = stgs[bi % 2]
                nc.scalar.copy(
                    stg[:, k * TILE:(k + 1) * TILE, 0:1]
                    .rearrange("p e one -> p (e one)"), g[:])
                if taps and l == 0 and t < 4:
                    gt = smallp.tile([128, TILE], f32, tag="zo")
                    nc.vector.tensor_copy(gt[:], g[:])
                    nc.sync.dma_start(g_o.ap()[:, t * TILE:(t + 1) * TILE],
                                      gt[:])
                if k == TPB - 1:
                    if taps and l == 0:
                        nc.sync.dma_start(
                            stg_o.ap()[:, bi * BATCH * 2:(bi + 1) * BATCH * 2],
                            stg.rearrange("p e two -> p (e two)"))
                    view = agg[:, par:par + 2 * NELEMS].rearrange(
                        "p (e two) -> p e two", two=2)
                    nc.gpsimd.scatter_add(
                        view, idxt[:, bi * IPB:(bi + 1) * IPB], stg[:],
                        channels=128, num_elems=NELEMS, d=2, num_idxs=BATCH)

            # ---- GIN MLP (feature-major) --------------------------------
            if taps and l == 0:
                nc.sync.dma_start(agg_o.ap(), agg[:])
            h = zfm_nxt
            nc.vector.tensor_add(h[:], agg[:, 0:PER_CORE], zfm_cur[:])
            for s0 in range(0, PER_CORE, TILE):
                s1 = min(s0 + TILE, PER_CORE)
                sw = s1 - s0
                p1 = mlpp.tile([128, TILE], f32, tag="p1")
                nc.tensor.matmul(p1[:, 0:sw], lhsT=w1[:, l * 128:(l + 1) * 128],
                                 rhs=h[:, s0:s1], start=True, stop=True)
                h1 = smallp.tile([128, TILE], bf16, tag="h1")
                nc.scalar.activation(h1[:, 0:sw], p1[:, 0:sw], Relu,
                                     bias=b1[:, l:l + 1])
                p2 = mlpp.tile([128, TILE], f32, tag="p1")
                nc.tensor.matmul(p2[:, 0:sw], lhsT=w2[:, l * 128:(l + 1) * 128],
                                 rhs=h1[:, 0:sw], start=True, stop=True)
                if l < L - 1:
                    nc.scalar.activation(h[:, s0:s1], p2[:, 0:sw], Relu,
                                         bias=b2[:, l:l + 1])
                else:
                    zo = smallp.tile([128, TILE], f32, tag="zo")
                    nc.scalar.activation(zo[:, 0:sw], p2[:, 0:sw], Relu,
                                         bias=b2[:, l:l + 1])
                    nc.sync.dma_start(
                        zout_t.ap()[:, s0:s1], zo[:, 0:sw])

            if taps and l == 0:
                nc.sync.dma_start(z1_o.ap(), h[:])

            # ---- z_next -> node-major + halo ----------------------------
            if l < L - 1:
                for g0 in range(0, NWC, 4):
                    gn = min(4, NWC - g0)
                    tp = tpp.tile([128, TILE], bf16, tag="tp")
                    for j in range(gn):
                        nc.tensor.transpose(
                            tp[:, j * 128:(j + 1) * 128],
                            h[:, (g0 + j) * 128:(g0 + j + 1) * 128],
                            ident[:])
                    zt = smallp.tile([128, TILE], bf16, tag="h1")
                    nc.scalar.copy(zt[:, 0:gn * 128], tp[:, 0:gn * 128])
                    nc.sync.dma_start(
                        zblk[l][:, g0 * 128:(g0 + gn) * 128],
                        zt[:, 0:gn * 128])
                if collectives:
                    nc.gpsimd.collective_compute(
                        "AllGather", mybir.AluOpType.bypass,
                        replica_groups=rg,
                        ins=[zblk[l].opt()], outs=[zsh[l].opt()])
                else:
                    nc.sync.dma_start(
                        zsh[l].rearrange("(r p) n -> r p n", r=NCORES)[0],
                        zblk[l][:])
                for r in range(NCORES):
                    nc.sync.dma_start(
                        zall[r].rearrange("p w d -> p (w d)"),
                        zsh[l][r * 128:(r + 1) * 128, :])

    nc.compile()
    return nc


def _make_in_maps(inputs, geom, percore):
    import ml_dtypes
    bf = ml_dtypes.bfloat16
    x = np.asarray(inputs["x"], np.float32)
    Ws1 = np.asarray(inputs["Ws1"], np.float32)
    bs1 = np.asarray(inputs["bs1"], np.float32)
    Ws2 = np.asarray(inputs["Ws2"], np.float32)
    bs2 = np.asarray(inputs["bs2"], np.float32)

    xp = np.zeros((NPAD, D), np.float32)
    xp[:N] = x
    zall0 = np.ascontiguousarray(
        xp.reshape(NW, 128, D).transpose(1, 0, 2).reshape(128, NW * D)
    ).astype(bf)
    srcw_all, idxt_all, nblk = _pack_tables(geom, percore)
    emat = np.zeros((128, EBLK, 128), np.float32)
    for k in range(EBLK):
        emat[k, k, :] = 1.0
    emat = emat.reshape(128, EBLK * 128).astype(bf)
    iota = np.arange(128, dtype=np.float32).reshape(128, 1)
    ident = np.eye(128, dtype=np.float32).astype(bf)
    w1 = np.concatenate([Ws1[l] for l in range(L)], axis=1).astype(bf)
    w2 = np.concatenate([Ws2[l] for l in range(L)], axis=1).astype(bf)
    b1 = np.ascontiguousarray(bs1.T).astype(np.float32)
    b2 = np.ascontiguousarray(bs2.T).astype(np.float32)

    in_maps = []
    for c in range(NCORES):
        zfm0 = np.ascontiguousarray(
            xp[c * PER_CORE:(c + 1) * PER_CORE].T).astype(bf)
        in_maps.append({
            "zall0": zall0, "zfm0": zfm0,
            "srcw": srcw_all[c].astype(bf),
            "emat": emat, "iota": iota, "ident": ident,
            "idxt": idxt_all[c].astype(np.int16),
            "w1": w1, "w2": w2, "b1": b1, "b2": b2,
        })
    return in_maps


def kernel(x, Ws1, bs1, Ws2, bs2, edge_index):
    geom, percore = _prepare_edges(edge_index)
    in_maps = _make_in_maps(
        {"x": x, "Ws1": Ws1, "bs1": bs1, "Ws2": Ws2, "bs2": bs2},
        geom, percore)
    nc = _build_program(geom)

    from concourse.bass_utils import run_bass_kernel_spmd
    res = run_bass_kernel_spmd(nc, in_maps, core_ids=list(range(NCORES)))
    global last_results
    last_results = res

    out = np.empty((NPAD, D), np.float32)
    for c in range(NCORES):
        out[c * PER_CORE:(c + 1) * PER_CORE] = res.results[c]["zout"].T
    return out[:N]


if __name__ == "__main__":
    data = np.load("/root/problem/inputs.npz")
    geom, percore = _prepare_edges(data["edge_index"])
    print("TOTC:", geom["TOTC"], "ntiles:", geom["ntiles"],
          "nbatch:", geom["nbatch"],
          "inflation:", geom["TOTC"] / (E / NCORES))
    nseg = sum(len(s) for s in geom["segs"])
    print("total matmul segments per layer:", nseg)
    out = _numpy_sim({k: data[k] for k in data.files}, geom, percore)
    exp = np.load("/root/problem/expected.npy")
    err = np.abs(out - exp).max() / np.abs(exp).max()
    print("numpy-sim rel err:", err)


# revision 29
# speedup vs baseline: 3.0453x; 1.0459x over previous
"""GIN encoder (3-layer, N=50000, E=800000, D=128) on 8 trn2 NeuronCores.

v2 strategy — descriptor-free aggregation (no dma_gather):
  - Every core keeps the FULL node-feature table Z in SBUF, node-major
    bf16 [128 slots, 392 windows, 128 feat] (all-gathered per layer).
  - Edges partitioned by dst core; per core the edge stream is grouped
    into cells (parity(dst), src window), padded uniformly across cores
    (SPMD). Per 512-column tile:
      1. PE "broadcast" matmul (one-hot lhsT E_k) replicates the tile's
         per-edge src-slot values from a packed [128, *] table to all
         128 partitions (PSUM fp32).
      2. DVE is_equal vs a per-partition iota builds the slot indicator
         [slot, col] in bf16.
      3. One PE matmul per (window-run in tile) gathers z[src] columns:
         G[feat, col] = Z_win^T_slotmajor @ indicator  (PSUM fp32).
      4. ACT copies G into a staging ring, bf16, stride-2 (d=2 layout
         with a permanent-zero partner slot).
      5. gpsimd.scatter_add accumulates staging into the feature-major
         agg [128, npairs, 2] (bf16), idx = dst node-pair; the odd-dst
         pass uses a one-column-shifted view of the same agg buffer.
    scatter_add loses duplicate updates within an aligned 8-index octet
    (SIMD width 8), so same-pair edges are round-robined across octets
    per cell at prep time; pad columns add 0 to a dump pair.
  - MLP runs feature-major on [128, 6272] (h = agg + z), then the own
    z_next is PE-transposed to node-major, DMA'd to HBM and AllGathered
    for the next layer's Z table.
"""

import numpy as np

N = 50000
E = 800000
D = 128
L = 3
NCORES = 8
PER_CORE = 6272          # 49 * 128 dst nodes per core
NPAD = 50176             # 8 * 6272
NW = 392                 # global 128-node source windows
NWC = 49                 # windows per core
NPAIRS = 3136            # dst node pairs per core
TILE = 512               # column tile (one PSUM bank)
BATCH = 3584             # scatter_add batch = 7 tiles, %16 == 0
NELEMS = 3140            # scatter_add num_elems (3136 real + dump space)
DUMP = 3139              # dump pair for pad columns
PADV = 300.0             # src-slot value for pad columns (never matches)
EBLK = 32                # one-hot broadcast matrices E_0..E_63


def _prepare_edges(edge_index):
    """Build the uniform cell geometry + per-core tables.

    Returns (geom, percore) where geom has the shared static structure and
    percore the per-core srcvals/idx tables.
    """
    src = np.asarray(edge_index[0], dtype=np.int64)
    dst = np.asarray(edge_index[1], dtype=np.int64)

    core = dst // PER_CORE
    dloc = dst % PER_CORE
    par = dloc & 1
    w = src >> 7
    slot = src & 127
    pairv = np.where(par == 0, dloc >> 1, (dloc - 1) >> 1)

    # cell id per edge: (core, par, w)
    cell = (core * 2 + par) * NW + w
    ncells = NCORES * 2 * NW
    counts = np.bincount(cell, minlength=ncells)

    # max multiplicity of (cell, pair) — octet lower bound
    cp = cell * np.int64(NPAIRS) + pairv
    _, cp_counts = np.unique(cp, return_counts=True)
    cp_cell = np.unique(cp) // NPAIRS
    maxmult = np.zeros(ncells, np.int64)
    np.maximum.at(maxmult, cp_cell, cp_counts)

    # scatter_add loses duplicate-pair updates closer than ~4 idx columns
    # (two 8-lane units stream the columns with skew). Enforce SEP-position
    # separation between same-pair updates within a scatter batch.
    SEP = 80

    order = np.lexsort((pairv, cell))
    cell_s = cell[order]
    cell_starts = np.zeros(ncells + 1, np.int64)
    np.cumsum(counts, out=cell_starts[1:])

    # per-cell edge lists (sorted by pair) per core
    K = np.maximum((counts + 7) // 8, maxmult).reshape(NCORES, 2, NW).max(0)

    def place(K):
        """Greedy octet assignment honoring SEP. Returns (ok, needK,
        srcvals, idxvals, off, tot, base)."""
        P = K * 8
        off = np.zeros((2, NW), np.int64)
        tot = np.zeros(2, np.int64)
        for p in (0, 1):
            off[p] = np.cumsum(np.concatenate([[0], P[p][:-1]]))
            tot[p] = int(np.ceil(P[p].sum() / BATCH)) * BATCH
        base = np.array([0, tot[0]], np.int64)
        TOTC = int(tot.sum())
        srcvals = np.full((NCORES, TOTC), PADV, np.float64)
        idxvals = np.full((NCORES, TOTC), DUMP, np.int64)
        needK = K.copy()
        ok = True
        sepo = (SEP + 7) // 8
        for c in range(NCORES):
            for p in (0, 1):
                nextpos = {}
                for wi in range(NW):
                    cid = (c * 2 + p) * NW + wi
                    s0, s1 = cell_starts[cid], cell_starts[cid + 1]
                    if s0 == s1:
                        continue
                    kk = int(K[p, wi])
                    cbase = int(base[p] + off[p, wi])
                    cap = np.zeros(kk, np.int64)
                    fill = [[] for _ in range(kk)]
                    eidx = order[s0:s1]
                    prs = pairv[eidx]
                    # groups by pair, biggest first
                    upr, inv, cnt = np.unique(prs, return_inverse=True,
                                              return_counts=True)
                    gorder = np.argsort(-cnt)
                    failed = False
                    for gi in gorder:
                        members = eidx[inv == gi]
                        pr = int(upr[gi])
                        for e in members:
                            o0 = nextpos.get(pr, -10**9)
                            o0 = max(0, (o0 - cbase + 7) // 8)
                            o = o0
                            while o < kk and cap[o] >= 8:
                                o += 1
                            if o >= kk:
                                failed = True
                                needK[p, wi] = max(needK[p, wi],
                                                   kk + max(1, o0 - kk + 1))
                                continue
                            fill[o].append(e)
                            cap[o] += 1
                            nextpos[pr] = cbase + 8 * o + 8 + SEP
                    if failed:
                        ok = False
                        continue
                    for o in range(kk):
                        for li, e in enumerate(fill[o]):
                            col = cbase + 8 * o + li
                            srcvals[c, col] = slot[e]
                            idxvals[c, col] = pairv[e]
        return ok, needK, srcvals, idxvals, off, tot, base

    for _ in range(6):
        ok, needK, srcvals, idxvals, off, tot, base = place(K)
        if ok:
            break
        K = needK
    assert ok, "greedy octet placement failed"
    P = K * 8
    TOTC = int(tot.sum())
    assert TOTC % BATCH == 0
    ntiles = TOTC // TILE

    # verify: same-pair separation >= SEP within each batch, octets distinct
    for c in range(NCORES):
        for p in (0, 1):
            covs = np.arange(base[p], base[p] + tot[p])
            idb = idxvals[c, covs].reshape(-1, BATCH)
            for b in range(idb.shape[0]):
                row = idb[b]
                real = row != DUMP
                pos = np.arange(BATCH)[real]
                prs = row[real]
                o = np.lexsort((pos, prs))
                same = prs[o][1:] == prs[o][:-1]
                gap = pos[o][1:] - pos[o][:-1]
                assert not (same & (gap < SEP)).any(), "separation violated"

    # tile segments: per tile, runs of (w, a, b) in-tile col ranges
    # (uniform across cores). Pad ranges use window 0 (indicator all-zero).
    bounds = []              # (colstart, colend, w) in stream order
    for p in (0, 1):
        cstart = base[p]
        for wi in range(NW):
            if P[p, wi]:
                s0 = base[p] + off[p, wi]
                bounds.append((s0, s0 + P[p, wi], wi))
        pe = base[p] + P[p].sum()
        if tot[p] > P[p].sum():
            bounds.append((pe, base[p] + tot[p], 0))
    segs = [[] for _ in range(ntiles)]
    for (s0, s1, wi) in bounds:
        t0, t1 = s0 // TILE, (s1 - 1) // TILE
        for t in range(t0, t1 + 1):
            a = max(s0, t * TILE) - t * TILE
            b = min(s1, (t + 1) * TILE) - t * TILE
            segs[t].append((wi, int(a), int(b)))

    geom = {
        "TOTC": TOTC, "ntiles": ntiles, "segs": segs,
        "tot": tot, "base": base,
        "nbatch": TOTC // BATCH,
        "par_of_tile": [0 if t * TILE < tot[0] else 1 for t in range(ntiles)],
    }
    percore = {"srcvals": srcvals, "idxvals": idxvals}
    return geom, percore


def _pack_tables(geom, percore):
    """srcW packing + wrapped scatter idx tables, per core."""
    TOTC, ntiles = geom["TOTC"], geom["ntiles"]
    nblk = (ntiles + EBLK - 1) // EBLK
    srcw = np.zeros((NCORES, 128, nblk * TILE), np.float64)
    sv = percore["srcvals"].reshape(NCORES, ntiles, TILE)
    for t in range(ntiles):
        srcw[:, t % EBLK, (t // EBLK) * TILE:(t // EBLK + 1) * TILE] = sv[:, t]

    idx = percore["idxvals"].astype(np.int16)    # [NCORES, TOTC]
    nb = geom["nbatch"]
    iw = idx.reshape(NCORES, nb, BATCH // 16, 16)
    idxt = np.tile(iw.transpose(0, 3, 1, 2).reshape(NCORES, 16, nb * (BATCH // 16)),
                   (1, 8, 1))                    # [NCORES, 128, nb*224]
    return srcw, idxt, nblk


def _numpy_sim(inputs, geom, percore):
    """Bit-approximate pipeline sim (fp32 math) to validate the tables."""
    x = np.asarray(inputs["x"], np.float32)
    Ws1, bs1 = np.asarray(inputs["Ws1"], np.float32), np.asarray(inputs["bs1"], np.float32)
    Ws2, bs2 = np.asarray(inputs["Ws2"], np.float32), np.asarray(inputs["bs2"], np.float32)
    xp = np.zeros((NPAD, D), np.float32)
    xp[:N] = x
    z = xp.copy()
    sv = percore["srcvals"]
    iv = percore["idxvals"]
    tot, base = geom["tot"], geom["base"]
    for l in range(L):
        zn = np.zeros_like(z)
        for c in range(NCORES):
            agg2 = np.zeros((D, NELEMS + 1, 2), np.float32)
            # gather G columns
            segs = geom["segs"]
            G = np.zeros((D, geom["TOTC"]), np.float32)
            for t, seglist in enumerate(segs):
                for (wi, a, b) in seglist:
                    cols = np.arange(t * TILE + a, t * TILE + b)
                    s = sv[c, cols]
                    real = s < 128
                    gsl = np.zeros((D, len(cols)), np.float32)
                    nodes = wi * 128 + s[real].astype(np.int64)
                    gsl[:, real] = z[nodes].T
                    G[:, cols] = gsl
            # scatter (true accumulation; octet constraint already asserted)
            for p in (0, 1):
                cols = np.arange(base[p], base[p] + tot[p])
                idxs = iv[c, cols]
                tgt = np.zeros((NELEMS + 1, D), np.float32)
                np.add.at(tgt, idxs, G[:, cols].T)
                agg2[:, :, p] += tgt.T
            # unpack agg2 -> agg cols: even pass wrote (pair k -> col 2k),
            # odd pass wrote (pair k -> col 2k+1)
            agg = np.zeros((D, PER_CORE), np.float32)
            agg[:, 0::2] = agg2[:, :NPAIRS, 0]
            agg[:, 1::2] = agg2[:, :NPAIRS, 1]
            zc = z[c * PER_CORE:(c + 1) * PER_CORE].T
            h = agg + zc
            h1 = np.maximum(Ws1[l].T @ h + bs1[l][:, None], 0)
            z2 = np.maximum(Ws2[l].T @ h1 + bs2[l][:, None], 0)
            zn[c * PER_CORE:(c + 1) * PER_CORE] = z2.T
        z = zn
    return z[:N]


def _build_program(geom, n_devices=NCORES, collectives=True, taps=False):
    import concourse.bacc as bacc
    import concourse.tile as tile
    import concourse.mybir as mybir
    from contextlib import ExitStack

    f32 = mybir.dt.float32
    bf16 = mybir.dt.bfloat16
    i16 = mybir.dt.int16
    Relu = mybir.ActivationFunctionType.Relu
    iseq = mybir.AluOpType.is_equal

    ntiles = geom["ntiles"]
    segs = geom["segs"]
    nb = geom["nbatch"]
    nblk = (ntiles + EBLK - 1) // EBLK
    TPB = BATCH // TILE          # tiles per scatter batch (7)
    IPB = BATCH // 16            # idx cols per batch (224)

    nc = bacc.Bacc("TRN2", debug=False, enable_asserts=False,
                   target_bir_lowering=False, num_devices=n_devices)

    zall0_t = nc.dram_tensor("zall0", [128, NW * 128], bf16, kind="ExternalInput")
    zfm0_t = nc.dram_tensor("zfm0", [128, PER_CORE], bf16, kind="ExternalInput")
    srcw_t = nc.dram_tensor("srcw", [128, nblk * TILE], bf16, kind="ExternalInput")
    emat_t = nc.dram_tensor("emat", [128, EBLK * 128], bf16, kind="ExternalInput")
    iota_t = nc.dram_tensor("iota", [128, 1], f32, kind="ExternalInput")
    ident_t = nc.dram_tensor("ident", [128, 128], bf16, kind="ExternalInput")
    idxt_t = nc.dram_tensor("idxt", [128, nb * IPB], i16, kind="ExternalInput")
    w1_t = nc.dram_tensor("w1", [128, L * 128], bf16, kind="ExternalInput")
    w2_t = nc.dram_tensor("w2", [128, L * 128], bf16, kind="ExternalInput")
    b1_t = nc.dram_tensor("b1", [128, L], f32, kind="ExternalInput")
    b2_t = nc.dram_tensor("b2", [128, L], f32, kind="ExternalInput")
    zout_t = nc.dram_tensor("zout", [128, PER_CORE], f32, kind="ExternalOutput")
    if taps:
        agg_o = nc.dram_tensor("agg_o", [128, 2 * NELEMS + 1], bf16,
                               kind="ExternalOutput")
        z1_o = nc.dram_tensor("z1_o", [128, PER_CORE], bf16,
                              kind="ExternalOutput")
        g_o = nc.dram_tensor("g_o", [128, 4 * TILE], f32,
                             kind="ExternalOutput")
        stg_o = nc.dram_tensor("stg_o", [128, geom["nbatch"] * BATCH * 2],
                               bf16, kind="ExternalOutput")

    rg = [list(range(NCORES))]

    with tile.TileContext(nc) as tc, ExitStack() as ctx:
        const = ctx.enter_context(tc.tile_pool(name="const", bufs=1))
        zap = ctx.enter_context(tc.tile_pool(name="za", bufs=1))
        zfp = ctx.enter_context(tc.tile_pool(name="zf", bufs=1))
        agp = ctx.enter_context(tc.tile_pool(name="ag", bufs=1))
        stp = ctx.enter_context(tc.tile_pool(name="st", bufs=1))
        indp = ctx.enter_context(tc.tile_pool(name="ind", bufs=2))
        smallp = ctx.enter_context(tc.tile_pool(name="sm", bufs=2))
        bcp = ctx.enter_context(tc.tile_pool(name="bc", bufs=2, space="PSUM"))
        gpp = ctx.enter_context(tc.tile_pool(name="gp", bufs=2, space="PSUM"))
        mlpp = ctx.enter_context(tc.tile_pool(name="mlp", bufs=2, space="PSUM"))
        tpp = ctx.enter_context(tc.tile_pool(name="tp", bufs=2, space="PSUM"))
        dram = ctx.enter_context(tc.tile_pool(name="dram", bufs=1, space="DRAM"))

        srcw = const.tile([128, nblk * TILE], bf16)
        emat = const.tile([128, EBLK * 128], bf16)
        iota = const.tile([128, 1], f32)
        ident = const.tile([128, 128], bf16)
        idxt = const.tile([128, nb * IPB], i16)
        w1 = const.tile([128, L * 128], bf16)
        w2 = const.tile([128, L * 128], bf16)
        b1 = const.tile([128, L], f32)
        b2 = const.tile([128, L], f32)
        for sb, t in ((srcw, srcw_t), (emat, emat_t), (iota, iota_t),
                      (ident, ident_t), (idxt, idxt_t), (w1, w1_t),
                      (w2, w2_t), (b1, b1_t), (b2, b2_t)):
            nc.sync.dma_start(sb[:], t.ap())

        zall = [zap.tile([128, NWC, 128], bf16, name=f"zall{r}")
                for r in range(NCORES)]
        for r in range(NCORES):
            nc.sync.dma_start(
                zall[r].rearrange("p w d -> p (w d)"),
                zall0_t.ap()[:, r * PER_CORE:(r + 1) * PER_CORE])
        zfmA = zfp.tile([128, PER_CORE], bf16)
        zfmB = zfp.tile([128, PER_CORE], bf16)
        nc.sync.dma_start(zfmA[:], zfm0_t.ap())
        agg = agp.tile([128, 2 * NELEMS + 1], bf16)
        stgs = [stp.tile([128, BATCH, 2], bf16, name=f"stg{i}") for i in (0, 1)]
        for s in stgs:
            nc.vector.memset(s.rearrange("p e two -> p (e two)"), 0.0)

        # node-major halo blocks: [128 slot-partitions, PER_CORE] per core;
        # AllGather concatenates along dim 0 -> [8*128, PER_CORE]
        zblk = [dram.tile([128, PER_CORE], bf16, name=f"zblk{l}", tag=f"zblk{l}")
                for l in range(L - 1)]
        sh = "Shared" if collectives else "Local"
        zsh = [dram.tile([NCORES * 128, PER_CORE], bf16, addr_space=sh,
                         name=f"zsh{l}", tag=f"zsh{l}") for l in range(L - 1)]

        for l in range(L):
            zfm_cur = zfmA if l % 2 == 0 else zfmB
            zfm_nxt = zfmB if l % 2 == 0 else zfmA
            nc.vector.memset(agg[:], 0.0)

            for t in range(ntiles):
                par = geom["par_of_tile"][t]
                bc = bcp.tile([128, TILE], f32, tag="bc")
                nc.tensor.matmul(
                    bc[:], lhsT=emat[:, (t % EBLK) * 128:(t % EBLK + 1) * 128],
                    rhs=srcw[:, (t // EBLK) * TILE:(t // EBLK + 1) * TILE],
                    start=True, stop=True)
                ind = indp.tile([128, TILE], bf16, tag="ind")
                nc.vector.tensor_tensor(
                    ind[:], iota[:].to_broadcast((128, TILE)), bc[:], op=iseq)
                g = gpp.tile([128, TILE], f32, tag="g")
                for (wi, a, b) in segs[t]:
                    nc.tensor.matmul(g[:, a:b],
                                     lhsT=zall[wi // NWC][:, wi % NWC, :],
                                     rhs=ind[:, a:b], start=True, stop=True)
                bi, k = divmod(t, TPB)
                stg = stgs[bi % 2]
                nc.scalar.copy(
                    stg[:, k * TILE:(k + 1) * TILE, 0:1]
                    .rearrange("p e one -> p (e one)"), g[:])
                if taps and l == 0 and t < 4:
                    gt = smallp.tile([128, TILE], f32, tag="zo")
                    nc.vector.tensor_copy(gt[:], g[:])
                    nc.sync.dma_start(g_o.ap()[:, t * TILE:(t + 1) * TILE],
                                      gt[:])
                if k == TPB - 1:
                    if taps and l == 0:
                        nc.sync.dma_start(
                            stg_o.ap()[:, bi * BATCH * 2:(bi + 1) * BATCH * 2],
                            stg.rearrange("p e two -> p (e two)"))
                    view = agg[:, par:par + 2 * NELEMS].rearrange(
                        "p (e two) -> p e two", two=2)
                    nc.gpsimd.scatter_add(
                        view, idxt[:, bi * IPB:(bi + 1) * IPB], stg[:],
                        channels=128, num_elems=NELEMS, d=2, num_idxs=BATCH)

            # ---- GIN MLP (feature-major) --------------------------------
            if taps and l == 0:
                nc.sync.dma_start(agg_o.ap(), agg[:])
            h = zfm_nxt
            nc.vector.tensor_add(h[:], agg[:, 0:PER_CORE], zfm_cur[:])
            for s0 in range(0, PER_CORE, TILE):
                s1 = min(s0 + TILE, PER_CORE)
                sw = s1 - s0
                p1 = mlpp.tile([128, TILE], f32, tag="p1")
                nc.tensor.matmul(p1[:, 0:sw], lhsT=w1[:, l * 128:(l + 1) * 128],
                                 rhs=h[:, s0:s1], start=True, stop=True)
                h1 = smallp.tile([128, TILE], bf16, tag="h1")
                nc.scalar.activation(h1[:, 0:sw], p1[:, 0:sw], Relu,
                                     bias=b1[:, l:l + 1])
                p2 = mlpp.tile([128, TILE], f32, tag="p1")
                nc.tensor.matmul(p2[:, 0:sw], lhsT=w2[:, l * 128:(l + 1) * 128],
                                 rhs=h1[:, 0:sw], start=True, stop=True)
                if l < L - 1:
                    nc.scalar.activation(h[:, s0:s1], p2[:, 0:sw], Relu,
                                         bias=b2[:, l:l + 1])
                else:
                    zo = smallp.tile([128, TILE], f32, tag="zo")
                    nc.scalar.activation(zo[:, 0:sw], p2[:, 0:sw], Relu,
                                         bias=b2[:, l:l + 1])
                    nc.sync.dma_start(
                        zout_t.ap()[:, s0:s1], zo[:, 0:sw])

            if taps and l == 0:
                nc.sync.dma_start(z1_o.ap(), h[:])

            # ---- z_next -> node-major + halo ----------------------------
            if l < L - 1:
                for g0 in range(0, NWC, 4):
                    gn = min(4, NWC - g0)
                    tp = tpp.tile([128, TILE], bf16, tag="tp")
                    for j in range(gn):
                        nc.tensor.transpose(
                            tp[:, j * 128:(j + 1) * 128],
                            h[:, (g0 + j) * 128:(g0 + j + 1) * 128],
                            ident[:])
                    zt = smallp.tile([128, TILE], bf16, tag="h1")
                    nc.scalar.copy(zt[:, 0:gn * 128], tp[:, 0:gn * 128])
                    nc.sync.dma_start(
                        zblk[l][:, g0 * 128:(g0 + gn) * 128],
                        zt[:, 0:gn * 128])
                if collectives:
                    nc.gpsimd.collective_compute(
                        "AllGather", mybir.AluOpType.bypass,
                        replica_groups=rg,
                        ins=[zblk[l].opt()], outs=[zsh[l].opt()])
                else:
                    nc.sync.dma_start(
                        zsh[l].rearrange("(r p) n -> r p n", r=NCORES)[0],
                        zblk[l][:])
                for r in range(NCORES):
                    nc.sync.dma_start(
                        zall[r].rearrange("p w d -> p (w d)"),
                        zsh[l][r * 128:(r + 1) * 128, :])

    nc.compile()
    return nc


def _make_in_maps(inputs, geom, percore):
    import ml_dtypes
    bf = ml_dtypes.bfloat16
    x = np.asarray(inputs["x"], np.float32)
    Ws1 = np.asarray(inputs["Ws1"], np.float32)
    bs1 = np.asarray(inputs["bs1"], np.float32)
    Ws2 = np.asarray(inputs["Ws2"], np.float32)
    bs2 = np.asarray(inputs["bs2"], np.float32)

    xp = np.zeros((NPAD, D), np.float32)
    xp[:N] = x
    zall0 = np.ascontiguousarray(
        xp.reshape(NW, 128, D).transpose(1, 0, 2).reshape(128, NW * D)
    ).astype(bf)
    srcw_all, idxt_all, nblk = _pack_tables(geom, percore)
    emat = np.zeros((128, EBLK, 128), np.float32)
    for k in range(EBLK):
        emat[k, k, :] = 1.0
    emat = emat.reshape(128, EBLK * 128).astype(bf)
    iota = np.arange(128, dtype=np.float32).reshape(128, 1)
    ident = np.eye(128, dtype=np.float32).astype(bf)
    w1 = np.concatenate([Ws1[l] for l in range(L)], axis=1).astype(bf)
    w2 = np.concatenate([Ws2[l] for l in range(L)], axis=1).astype(bf)
    b1 = np.ascontiguousarray(bs1.T).astype(np.float32)
    b2 = np.ascontiguousarray(bs2.T).astype(np.float32)

    in_maps = []
    for c in range(NCORES):
        zfm0 = np.ascontiguousarray(
            xp[c * PER_CORE:(c + 1) * PER_CORE].T).astype(bf)
        in_maps.append({
            "zall0": zall0, "zfm0": zfm0,
            "srcw": srcw_all[c].astype(bf),
            "emat": emat, "iota": iota, "ident": ident,
            "idxt": idxt_all[c].astype(np.int16),
            "w1": w1, "w2": w2, "b1": b1, "b2": b2,
        })
    return in_maps


def kernel(x, Ws1, bs1, Ws2, bs2, edge_index):
    geom, percore = _prepare_edges(edge_index)
    in_maps = _make_in_maps(
        {"x": x, "Ws1": Ws1, "bs1": bs1, "Ws2": Ws2, "bs2": bs2},
        geom, percore)
    nc = _build_program(geom)

    from concourse.bass_utils import run_bass_kernel_spmd
    res = run_bass_kernel_spmd(nc, in_maps, core_ids=list(range(NCORES)))
    global last_results
    last_results = res

    out = np.empty((NPAD, D), np.float32)
    for c in range(NCORES):
        out[c * PER_CORE:(c + 1) * PER_CORE] = res.results[c]["zout"].T
    return out[:N]


if __name__ == "__main__":
    data = np.load("/root/problem/inputs.npz")
    geom, percore = _prepare_edges(data["edge_index"])
    print("TOTC:", geom["TOTC"], "ntiles:", geom["ntiles"],
          "nbatch:", geom["nbatch"],
          "inflation:", geom["TOTC"] / (E / NCORES))
    nseg = sum(len(s) for s in geom["segs"])
    print("total matmul segments per layer:", nseg)
    out = _numpy_sim({k: data[k] for k in data.files}, geom, percore)
    exp = np.load("/root/problem/expected.npy")
    err = np.abs(out - exp).max() / np.abs(exp).max()
    print("numpy-sim rel err:", err)
